# revision 50
# baseline (speedup 1.0000x reference)
"""BiMamba block (bidirectional Mamba-1 + residual + LayerNorm) on 8 TRN2
NeuronCores.

Sharding: data-parallel over batch (B=8 -> one batch element per core).
Each core runs both directions for its batch element; no collectives.

Two compiled paths, selected at run time by an exact host-side check
(_fast_prep):

FAST PATH (banded low-rank kernel; used when A_log rows are identical
and dt is near-constant over time per channel -- both verified on the
host against the actual inputs, with exact-dt recomputation):
  The selective scan y[d,t] = sum_n C[n,t] h[d,n,t] collapses to
  y[d,t] = sum_{s<=t} K_d[t,s] v[d,s] with a rank-N kernel
  K_d[t,s] = sum_n B[n,s] C[n,t] e^{A_n u_d (t-s)} that depends on the
  channel only through the scalar mean step u_d = softplus(dt_b[d]).
  Channels are sorted by u_d on the host (weights permuted; out_w rows
  permuted back), so each 128-channel group shares one kernel built at
  the group's Chebyshev midpoint node (FM=1; rel interp error ~1e-6,
  far below bf16 noise).  The kernel is banded (decay cutoff, window
  FW=384 per 128-row s-tile) and built on the PE as 16-contraction
  matmuls from table-scaled B/C windows; z accumulates banded window
  matmuls into PSUM [128ch, L].  The DVE scan, the exp(dt*A) ACT
  volume, and the w/hC elementwise volumes all disappear; the whole
  SSM becomes PE matmul work.  Measured ~2.4e-4 rel err, ~451 us on HW
  (2.6x over the v2 scan kernel).
  Engine split: PE in-proj/dbc/dt/vT-transposes/K-build/z/out-proj/
  combine; DVE depthwise conv (per-partition tap scalars), v=dt*xc,
  table scalings, K diagonal causal masks, gates, LN stats; ACT silu/
  softplus/PSUM evictions; GPSIMD idle (it shares the DVE SBUF port --
  offloading there slows the DVE).

FALLBACK (exact selective scan, any inputs): the v2 design -- DVE
tensor_tensor_scan over [P, 4 states x L] streams, ACT exp volume, PE
identity-accumulate state reduction, with hoisted dt chains and
deferred gates.

Backward direction runs the same causal pipeline on the host-reversed
sequence; the combine stage un-reverses it with an anti-identity
matmul, then residual + LayerNorm in fp32 (PE transposes to
token-major, bn_stats/bn_aggr), output [L, DM] per core.
"""

import numpy as np
import ml_dtypes
from contextlib import ExitStack

L, DM, DI, N, R, KC = 1024, 512, 1024, 16, 32, 4
P = 128
HALF = 512          # matmul moving-operand / PSUM-bank chunk (fp32 out)
NDT = DI // P       # 8 d-tiles
NTT = L // P        # 8 token tiles
NKT = DM // P       # 4 dm k-tiles

# fast (banded low-rank kernel) path constants
FM = 1              # Chebyshev nodes per sorted channel group
FW = 320            # K window width per s-tile (band 192..319; sharp
                    # band-192 cutoff validated at 1.8e-6 rel err in fp64)

_CACHE = {}


def _np_softplus(x):
    return np.log1p(np.exp(-np.abs(x))) + np.maximum(x, 0)


def _np_silu(x):
    return x / (1 + np.exp(-x))


def _fast_prep(inputs):
    """Host gate + per-direction tables for the fast path.

    The fast path replaces the sequential selective scan with banded
    low-rank kernel matmuls.  Valid when (a) A_log rows are identical
    (decay rates shared across channels) and (b) dt is nearly constant
    in time per channel (selectivity small), both checked here exactly.
    Returns None if the inputs don't qualify (kernel falls back to the
    exact scan path)."""
    x = np.asarray(inputs["x"], np.float32)
    prep = {}
    for s in ("f", "b"):
        al = np.asarray(inputs[f"A_log_{s}"])
        if al.shape != (DI, N) or np.ptp(al, axis=0).max() != 0.0:
            return None
        xs = x if s == "f" else x[:, ::-1]
        w_in = np.asarray(inputs[f"in_w_{s}"], np.float32)
        xi = xs.reshape(-1, DM) @ w_in[:, :DI]
        xi = xi.reshape(8, L, DI)
        xp = np.pad(xi, ((0, 0), (KC - 1, 0), (0, 0)))
        cw = np.asarray(inputs[f"conv_w_{s}"], np.float32)
        xc = sum(xp[:, k:k + L, :] * cw[:, k] for k in range(KC))
        xc = _np_silu(xc + np.asarray(inputs[f"conv_b_{s}"], np.float32))
        wx = np.asarray(inputs[f"xproj_w_{s}"], np.float32)
        dtarg = (xc.reshape(-1, DI) @ wx[:, :R]) @ \
            np.asarray(inputs[f"dt_w_{s}"], np.float32)
        dt = _np_softplus(dtarg.reshape(8, L, DI)
                          + np.asarray(inputs[f"dt_b_{s}"], np.float32))
        dbar = _np_softplus(np.asarray(inputs[f"dt_b_{s}"], np.float32))
        if np.abs(dt - dbar[None, None, :]).max() > 3e-3:
            return None
        perm = np.argsort(dbar, kind="stable")
        u = dbar[perm]
        A1 = -np.exp(al[0]).astype(np.float64)          # [N], shared
        tpos = np.zeros((FM, NDT, N, FW), np.float64)
        tneg = np.zeros((FM, NDT, N, P), np.float64)
        phi = np.zeros((FM, DI), np.float64)
        tau = np.arange(FW, dtype=np.float64)
        sig = np.arange(P, dtype=np.float64)
        for D in range(NDT):
            ug = u[D * P:(D + 1) * P]
            lo, hi = float(ug.min()), float(ug.max())
            if FM > 1:
                kk = np.arange(FM)
                um = (lo + hi) / 2 + (hi - lo) / 2 * np.cos(
                    (2 * kk + 1) * np.pi / (2 * FM))
            else:
                um = np.array([(lo + hi) / 2])
            for m in range(FM):
                pm = np.ones(P)
                for m2 in range(FM):
                    if m2 != m:
                        pm *= (ug - um[m2]) / (um[m] - um[m2])
                phi[m, D * P:(D + 1) * P] = pm
                tpos[m, D] = np.exp(A1[:, None] * um[m] * tau[None, :])
                tneg[m, D] = np.exp(-A1[:, None] * um[m] * sig[None, :])
        bf = ml_dtypes.bfloat16
        prep[f"perm_{s}"] = perm
        prep[f"tpos_{s}"] = tpos.astype(bf)
        prep[f"tneg_{s}"] = tneg.astype(bf)
        prep[f"phi_{s}"] = phi.astype(bf)
    return prep


def _emit_stage_a(nc, tc, actx, dr, sfx, xin_name, fast=False):
    """Projections + conv for one direction. Returns the tensors the scan
    stage needs (persistent pool `pool` lives until the scan is done).
    fast=True: depthwise conv runs on the DVE (tensor_scalar + STT chain
    with per-partition tap weights) instead of PE diag matmuls."""
    from concourse import mybir
    AL = mybir.AluOpType
    AF = mybir.ActivationFunctionType
    F32, BF = mybir.dt.float32, mybir.dt.bfloat16

    pool = actx.enter_context(tc.tile_pool(name=f"dir_{sfx}", bufs=1))
    st = {"pool": pool}

    convb = pool.tile([P, NDT], F32, name=f"convb_{sfx}")
    nc.sync.dma_start(convb, dr[f"convb_{sfx}"])
    dtb = pool.tile([P, NDT], F32, name=f"dtb_{sfx}")
    nc.sync.dma_start(dtb, dr[f"dtb_{sfx}"])
    aneg = pool.tile([P, NDT, N], F32, name=f"aneg_{sfx}")
    nc.sync.dma_start(aneg, dr[f"aneg_{sfx}"])
    dvec = pool.tile([P, NDT], F32, name=f"dvec_{sfx}")
    nc.sync.dma_start(dvec, dr[f"dvec_{sfx}"])
    st["aneg"], st["dvec"] = aneg, dvec

    sz_dram = dr[f"szscratch_{sfx}"]
    xc_dram = dr[f"xcscratch_{sfx}"]

    wdt = pool.tile([R, DI], BF, name=f"wdt_{sfx}")
    nc.sync.dma_start(wdt, dr[f"wdt_{sfx}"])
    st["wdt"] = wdt

    with ExitStack() as sctx:
        apool = sctx.enter_context(tc.tile_pool(name=f"stgA_{sfx}", bufs=1))
        atmp = sctx.enter_context(tc.tile_pool(name=f"stgAt_{sfx}", bufs=3))
        psA = sctx.enter_context(
            tc.tile_pool(name=f"psA_{sfx}", bufs=2, space="PSUM"))

        if fast:
            convsc = apool.tile([P, NDT, KC], F32, name=f"convsc_{sfx}")
            nc.sync.dma_start(convsc, dr[f"convsc_{sfx}"])
        else:
            convdiag = apool.tile([P, NDT * KC * P], BF,
                                  name=f"convdiag_{sfx}")
            nc.sync.dma_start(convdiag, dr[f"convdiag_{sfx}"])
        wx = apool.tile([P, NDT, R + 2 * N], BF, name=f"wx_{sfx}")
        nc.sync.dma_start(wx, dr[f"wx_{sfx}"])
        xT = apool.tile([P, NKT, L], BF, name=f"xT_{sfx}")
        nc.sync.dma_start(xT, dr[xin_name])

        # in-projection xz = x @ Win, xi and z halves interleaved per
        # channel so the PE has ~6us of matmul work per channel while the
        # DVE conv chain (~3us) trails without stalling anything.
        xc_sb = []
        winh = apool.tile([P, NKT, DI], BF, tag="win1", name=f"win_{sfx}")
        nc.sync.dma_start(winh, dr[f"win_{sfx}"][:, :, 0:DI])
        winh2 = apool.tile([P, NKT, DI], BF, tag="win2", name=f"win2_{sfx}")
        for ch in range(NDT):
            ps = psA.tile([P, L], F32, tag="psA", name="psxz")
            for h in range(2):
                for kt in range(NKT):
                    nc.tensor.matmul(
                        ps[:, h * HALF:(h + 1) * HALF],
                        lhsT=winh[:, kt, ch * P:(ch + 1) * P],
                        rhs=xT[:, kt, h * HALF:(h + 1) * HALF],
                        start=(kt == 0), stop=(kt == NKT - 1))
            if ch == 0:
                # defer the z-half weight DMA so it never delays the
                # first xi matmuls (it's only needed at z(0), ~6us later)
                nc.sync.dma_start(winh2, dr[f"win_{sfx}"][:, :, DI:2 * DI])
            xi = atmp.tile([P, L + KC - 1], BF, tag="xi", name="xi")
            nc.vector.memset(xi[:, 0:KC - 1], 0.0)
            nc.scalar.activation(xi[:, KC - 1:], ps, AF.Copy)
            if fast:
                # depthwise causal conv on DVE: per-tap tensor_scalar
                # (4x mode) + a 2x tensor_tensor add tree.
                tks = []
                for k in range(KC):
                    tk = atmp.tile([P, L], BF, tag=f"cvt{k}", name="cvt",
                                   bufs=2)
                    nc.vector.tensor_scalar_mul(
                        tk, xi[:, k:k + L], convsc[:, ch, k:k + 1])
                    tks.append(tk)
                s01 = atmp.tile([P, L], BF, tag="cvs0", name="cvs0", bufs=2)
                nc.vector.tensor_tensor(s01, tks[0], tks[1], AL.add)
                s23 = atmp.tile([P, L], BF, tag="cvs1", name="cvs1", bufs=2)
                nc.vector.tensor_tensor(s23, tks[2], tks[3], AL.add)
                acc = atmp.tile([P, L], BF, tag="cvs2", name="cvs2", bufs=2)
                nc.vector.tensor_tensor(acc, s01, s23, AL.add)
                t = apool.tile([P, L], BF, tag=f"xc{ch}", name=f"xc{ch}")
                nc.scalar.activation(t, acc, AF.Silu,
                                     bias=convb[:, ch:ch + 1])
            else:
                # depthwise causal conv (4 taps) on the PE via diagonal
                # tap matrices over shifted windows.
                cps = psA.tile([P, L], F32, tag="psA", name="pscv")
                for h in range(2):
                    for k in range(KC):
                        nc.tensor.matmul(
                            cps[:, h * HALF:(h + 1) * HALF],
                            lhsT=convdiag[:, (ch * KC + k) * P:
                                          (ch * KC + k + 1) * P],
                            rhs=xi[:, k + h * HALF:k + (h + 1) * HALF],
                            start=(k == 0), stop=(k == KC - 1))
                t = apool.tile([P, L], BF, tag=f"xc{ch}", name=f"xc{ch}")
                nc.scalar.activation(t, cps, AF.Silu,
                                     bias=convb[:, ch:ch + 1])
            xc_sb.append(t)
            ps2 = psA.tile([P, L], F32, tag="psA", name="psz")
            for h in range(2):
                for kt in range(NKT):
                    nc.tensor.matmul(
                        ps2[:, h * HALF:(h + 1) * HALF],
                        lhsT=winh2[:, kt, ch * P:(ch + 1) * P],
                        rhs=xT[:, kt, h * HALF:(h + 1) * HALF],
                        start=(kt == 0), stop=(kt == NKT - 1))
            tz = atmp.tile([P, L], BF, tag="sz", name="sz")
            nc.scalar.activation(tz, ps2, AF.Silu)
            nc.sync.dma_start(sz_dram[ch], tz)

        # x-projection: dbc = xc @ Wx  [R+2N, L] channel-major; keep the
        # SBUF copy persistent (dt is re-derived from it per scan block)
        # and park a DRAM copy for the B/C broadcast DMAs.
        dbc_ps = psA.tile([R + 2 * N, L], F32, tag="psA", name="psdbc")
        for h in range(2):
            for kt in range(NDT):
                nc.tensor.matmul(
                    dbc_ps[:, h * HALF:(h + 1) * HALF],
                    lhsT=wx[:, kt, :],
                    rhs=xc_sb[kt][:, h * HALF:(h + 1) * HALF],
                    start=(kt == 0), stop=(kt == NDT - 1))
        dbc = pool.tile([R + 2 * N, L], BF, name=f"dbc_{sfx}")
        nc.scalar.activation(dbc, dbc_ps, AF.Copy)
        nc.sync.dma_start(dr[f"dbcscratch_{sfx}"], dbc)
        st["dbc"] = dbc

        # park xc to DRAM (re-read at scan time for v = dt*xc + evict)
        for d in range(NDT):
            nc.sync.dma_start(xc_dram[d], xc_sb[d])

    st["dtb"] = dtb
    return st


def _emit_reps(nc, tc, rctx, dr, sfx, st):
    """B/C broadcast super-tiles [P, 4, L]: 4 states per tile via DMA
    partition-broadcast from the DRAM copy of dbc."""
    from concourse import mybir
    import concourse.bass as bass
    BF = mybir.dt.bfloat16

    rpool = rctx.enter_context(tc.tile_pool(name=f"reps_{sfx}", bufs=1))
    brep4, crep4 = [], []
    dbcd = dr[f"dbcscratch_{sfx}"]
    for gi, lst in ((0, brep4), (1, crep4)):
        for g in range(N // 4):
            t = rpool.tile([P, 4, L], BF, name=f"rep{gi}_{g}")
            for j in range(4):
                row = dbcd[R + gi * N + g * 4 + j:
                           R + gi * N + g * 4 + j + 1, :]
                nc.sync.dma_start(out=t[:, j, :], in_=bass.AP(
                    tensor=row.tensor, offset=row.offset,
                    ap=[[0, P]] + row.ap[1:]))
            lst.append(t)
    st.update(brep4=brep4, crep4=crep4)


def _emit_dt_phase(nc, tc, dctx, dr, sfx, st, onep):
    """Hoisted dt computation for all 8 d-tiles of one direction: the PE
    matmuls + ACT softplus chains run ahead of the other direction's
    stage A, so the scans never wait on a cold PE queue.  Also groups all
    Exp/Ln ACT ops (one act-table load per direction)."""
    from concourse import mybir
    AF = mybir.ActivationFunctionType
    F32, BF = mybir.dt.float32, mybir.dt.bfloat16

    wdt, dbc, dtb = st["wdt"], st["dbc"], st["dtb"]
    dtpool = dctx.enter_context(tc.tile_pool(name=f"dtph_{sfx}", bufs=1))
    dts = []
    with ExitStack() as pctx:
        psD = pctx.enter_context(
            tc.tile_pool(name=f"psD_{sfx}", bufs=2, space="PSUM"))
        for d in range(NDT):
            dtps = psD.tile([P, L], F32, tag="psdt", name="psdt")
            for h in range(2):
                nc.tensor.matmul(
                    dtps[:, h * HALF:(h + 1) * HALF],
                    lhsT=wdt[:, d * P:(d + 1) * P],
                    rhs=dbc[0:R, h * HALF:(h + 1) * HALF],
                    start=True, stop=True)
            dtmid = dtpool.tile([P, L], BF, tag="dtm", name="dtm", bufs=2)
            nc.scalar.activation(dtmid, dtps, AF.Exp, bias=dtb[:, d:d + 1])
            dt_d = dtpool.tile([P, L], BF, name=f"dtd{d}")
            nc.scalar.activation(dt_d, dtmid, AF.Ln, bias=onep)
            dts.append(dt_d)
    st["dts"] = dts


def _emit_scan(nc, tc, dctx, dr, sfx, st, ibf, onep):
    """Selective scan + gating + out-projection for one direction.
    All elementwise work on DVE (GPSIMD shares the DVE SBUF port and
    slows the scans); PE accumulates the state-reduce.  Gate ops for
    d-tile k are deferred until after d-tile k+1's scans so the DVE
    never stalls waiting on the PE reduce."""
    from concourse import mybir
    import concourse.bass as bass
    AL = mybir.AluOpType
    AF = mybir.ActivationFunctionType
    F32, BF = mybir.dt.float32, mybir.dt.bfloat16

    brep4, crep4 = st["brep4"], st["crep4"]
    aneg, dvec = st["aneg"], st["dvec"]
    dts, dtb = st["dts"], st["dtb"]

    def rep4(ap2d):
        return bass.AP(tensor=ap2d.tensor, offset=ap2d.offset,
                       ap=[ap2d.ap[0], [0, 4]] + ap2d.ap[1:])

    yg_sb = []
    tmp2 = dctx.enter_context(tc.tile_pool(name=f"tmp_{sfx}", bufs=2))
    scanp = dctx.enter_context(tc.tile_pool(name=f"scan_{sfx}", bufs=3))
    psY = dctx.enter_context(
        tc.tile_pool(name=f"psY_{sfx}", bufs=2, space="PSUM"))

    pend = []   # deferred gate work: (d, yps, xcr, szr)

    def emit_gate(d, yps, xcr, szr):
        yd = tmp2.tile([P, L], BF, tag="yd", name="yd")
        nc.vector.scalar_tensor_tensor(
            yd, xcr, dvec[:, d:d + 1], yps, AL.mult, AL.add)
        t = tmp2.tile([P, L], BF, tag=f"yg{d}", name=f"yg{d}", bufs=1)
        nc.vector.tensor_tensor(t, yd, szr, AL.mult)
        yg_sb.append(t)

    for d in range(NDT):
        dt_d = dts[d]
        xcr = tmp2.tile([P, L], BF, tag="xcr", name="xcr", bufs=2)
        nc.sync.dma_start(xcr, dr[f"xcscratch_{sfx}"][d])
        szr = tmp2.tile([P, L], BF, tag="szr", name="szr", bufs=2)
        nc.sync.dma_start(szr, dr[f"szscratch_{sfx}"][d])
        v_d = tmp2.tile([P, L], BF, tag="vd", name="vd", bufs=1)
        nc.vector.tensor_tensor(v_d, dt_d, xcr, AL.mult)
        yps = psY.tile([P, L], F32, tag="psY", name="psy")
        for g in range(N // 4):
            # w4/h4 at bufs=1 is stall-free: their producer/consumer pairs
            # are adjacent in DVE program order anyway.
            w4 = scanp.tile([P, 4, L], BF, tag="w4", name="w4", bufs=1)
            nc.vector.tensor_tensor(w4, rep4(v_d), brep4[g], AL.mult)
            # a = exp(dt*A_n); a=0 at each state's t=0 resets the carried
            # state exactly (h = 0*h_prev + w[0]).
            a4 = scanp.tile([P, 4, L], BF, tag="a4", name="a4", bufs=2)
            for j in range(4):
                n = g * 4 + j
                nc.scalar.activation(
                    a4[:, j, :], dt_d, AF.Exp,
                    scale=aneg[:, d, n:n + 1])
            nc.vector.memset(a4[:, 1:4, 0:1], 0.0)
            h4 = scanp.tile([P, 4, L], BF, tag="h4", name="h4", bufs=1)
            nc.vector.tensor_tensor_scan(
                h4.rearrange("p a b -> p (a b)"),
                a4.rearrange("p a b -> p (a b)"),
                w4.rearrange("p a b -> p (a b)"), 0.0, AL.mult, AL.add)
            hc4 = scanp.tile([P, 4, L], BF, tag="hc4", name="hc4", bufs=2)
            nc.vector.tensor_tensor(hc4, h4, crep4[g], AL.mult)
            for j in range(4):
                for h in range(2):
                    nc.tensor.matmul(
                        yps[:, h * HALF:(h + 1) * HALF],
                        lhsT=ibf,
                        rhs=hc4[:, j, h * HALF:(h + 1) * HALF],
                        start=(g == 0 and j == 0),
                        stop=(g == N // 4 - 1 and j == 3))
        # evict: yg = (y + xc*D) * silu(z) — deferred one d-tile so the
        # DVE's queue head never blocks on the PE reduce of this d-tile.
        pend.append((d, yps, xcr, szr))
        if len(pend) > 1:
            dd, yy, xx, ss = pend.pop(0)
            emit_gate(dd, yy, xx, ss)
    while pend:
        dd, yy, xx, ss = pend.pop(0)
        emit_gate(dd, yy, xx, ss)

    # out-projection: ydm = yg @ Wout  [DM, L] dm-major, f32 -> DRAM
    for mt in range(NKT):
        wout = tmp2.tile([P, NDT, P], BF, tag="wout", name="wout", bufs=2)
        nc.sync.dma_start(wout, dr[f"wout_{sfx}"][:, :, mt * P:(mt + 1) * P])
        ps = psY.tile([P, L], F32, tag="psY", name="psydm")
        for h in range(2):
            for kt in range(NDT):
                nc.tensor.matmul(
                    ps[:, h * HALF:(h + 1) * HALF],
                    lhsT=wout[:, kt, :],
                    rhs=yg_sb[kt][:, h * HALF:(h + 1) * HALF],
                    start=(kt == 0), stop=(kt == NDT - 1))
        t = tmp2.tile([P, L], F32, tag="ydmout", name="ydmout", bufs=1)
        nc.scalar.activation(t, ps, AF.Copy)
        nc.sync.dma_start(dr[f"ydmscratch_{sfx}"][mt], t)


def _emit_fast_ssm(nc, tc, dctx, dr, sfx, st, ibf, maskd, zcol, zrow):
    """Banded low-rank kernel path for one direction (replaces the scan).

    y[d,t] = sum_{s<=t, t-s<=band} K_D[s,t] * v[d,s] per sorted channel
    group D, where K_D[s,t] = sum_n B[n,s] C[n,t] e^{A_n u_D (t-s)} is a
    rank-N kernel shared by the group's 128 channels (u_D = group-center
    mean dt).  Per s-tile k the kernel occupies a [128, FW] window built
    as one 16-contraction matmul from table-scaled B/C; z accumulates
    window matmuls into PSUM [128d, L]."""
    from concourse import mybir
    import concourse.bass as bass
    AL = mybir.AluOpType
    AF = mybir.ActivationFunctionType
    F32, BF = mybir.dt.float32, mybir.dt.bfloat16

    dts, dvec = st["dts"], st["dvec"]
    dbcd = dr[f"dbcscratch_{sfx}"]
    fpool = dctx.enter_context(tc.tile_pool(name=f"fs_{sfx}", bufs=1))
    tmp = dctx.enter_context(tc.tile_pool(name=f"ft_{sfx}", bufs=2))
    kpool = dctx.enter_context(tc.tile_pool(name=f"fk_{sfx}", bufs=2))

    # ---------------- phase 1: v, then vT via PE transposes -------------
    vT = fpool.tile([P, NTT, DI], BF, name=f"vT_{sfx}")
    with ExitStack() as tctx:
        psT = tctx.enter_context(
            tc.tile_pool(name=f"psT_{sfx}", bufs=2, space="PSUM"))
        for d in range(NDT):
            xcr = tmp.tile([P, L], BF, tag="xcr", name="xcr", bufs=2)
            nc.sync.dma_start(xcr, dr[f"xcscratch_{sfx}"][d])
            v_d = tmp.tile([P, L], BF, tag="vd", name="vd", bufs=2)
            nc.vector.tensor_tensor(v_d, dts[d], xcr, AL.mult)
            for q in range(2):
                tp = psT.tile([P, 4, P], BF, tag="tp", name="tp")
                for j in range(4):
                    k = q * 4 + j
                    nc.tensor.transpose(
                        tp[:, j, :], v_d[:, k * P:(k + 1) * P], ibf)
                nc.scalar.activation(
                    vT[:, q * 4:(q + 1) * 4, d * P:(d + 1) * P], tp, AF.Copy)

    # ------------- phase 2: B/C windows (shared across groups) ----------
    cwin = fpool.tile([N, NTT, FW], BF, name=f"cwin_{sfx}")
    bwin = fpool.tile([N, NTT, P], BF, name=f"bwin_{sfx}")
    for k in range(NTT):
        cols = min(FW, L - P * k)
        nc.sync.dma_start(cwin[:, k, 0:cols],
                          dbcd[R + N:R + 2 * N, P * k:P * k + cols])
        nc.sync.dma_start(bwin[:, k, :],
                          dbcd[R:R + N, P * k:P * k + P])

    # ---------------- phase 3: per-group K build + z + gates ------------
    psKp = dctx.enter_context(
        tc.tile_pool(name=f"psK_{sfx}", bufs=1, space="PSUM"))
    psZp = dctx.enter_context(
        tc.tile_pool(name=f"psZ_{sfx}", bufs=1, space="PSUM"))

    def repk(ap2d, n):
        return bass.AP(tensor=ap2d.tensor, offset=ap2d.offset,
                       ap=[ap2d.ap[0], [0, n]] + ap2d.ap[1:])

    def build_K(D):
        ct, bt = {}, {}
        for m in range(FM):
            tpt = tmp.tile([N, FW], BF, tag=f"tp{m}", name="tpt", bufs=3)
            nc.sync.dma_start(tpt, dr[f"tpos_{sfx}"][m][D])
            tnt = tmp.tile([N, P], BF, tag=f"tn{m}", name="tnt", bufs=3)
            nc.sync.dma_start(tnt, dr[f"tneg_{sfx}"][m][D])
            c = tmp.tile([N, NTT, FW], BF, tag=f"ct{m}", name="ct", bufs=3)
            nc.vector.tensor_tensor(c, cwin, repk(tpt, NTT), AL.mult)
            b = tmp.tile([N, NTT, P], BF, tag=f"bt{m}", name="bt", bufs=3)
            nc.vector.tensor_tensor(b, bwin, repk(tnt, NTT), AL.mult)
            ct[m], bt[m] = c, b
        Ks = {}
        for m in range(FM):
            for k in range(NTT):
                cols = min(FW, L - P * k)
                psk = psKp.tile([P, FW], F32, tag="psk", name="psk", bufs=2)
                nc.tensor.matmul(
                    psk[:, 0:cols],
                    lhsT=bt[m][:, k, :],
                    rhs=ct[m][:, k, 0:cols],
                    start=True, stop=True)
                Kt = kpool.tile([P, FW], BF, tag=f"K{m}_{k}", name="Kt",
                                bufs=3)
                # full-window evict on ACT, then the causal mask as an
                # in-place 2x-mode DVE multiply on the SBUF diag block —
                # keeps the DVE off the per-D critical path (it runs in
                # lockstep with the PE otherwise).
                nc.scalar.activation(Kt[:, 0:cols], psk[:, 0:cols],
                                     AF.Copy)
                nc.vector.tensor_tensor(Kt[:, 0:P], Kt[:, 0:P], maskd,
                                        AL.mult)
                Ks[(m, k)] = Kt
        return Ks

    yg_sb = []
    pend = []

    def emit_gate(D, psz, xcr2, szr):
        yd = tmp.tile([P, L], BF, tag="yd", name="yd")
        nc.vector.scalar_tensor_tensor(
            yd, xcr2, dvec[:, D:D + 1], psz, AL.mult, AL.add)
        t = tmp.tile([P, L], BF, tag=f"yg{D}", name=f"yg{D}", bufs=1)
        nc.vector.tensor_tensor(t, yd, szr, AL.mult)
        yg_sb.append(t)

    Kq = [build_K(0), build_K(1)]
    for D in range(NDT):
        if D + 2 < NDT:
            Kq.append(build_K(D + 2))
        Ks = Kq.pop(0)
        xcr2 = tmp.tile([P, L], BF, tag="xcr2", name="xcr2", bufs=2)
        nc.sync.dma_start(xcr2, dr[f"xcscratch_{sfx}"][D])
        szr = tmp.tile([P, L], BF, tag="szr", name="szr", bufs=2)
        nc.sync.dma_start(szr, dr[f"szscratch_{sfx}"][D])
        psz = psZp.tile([P, L], F32, tag="psz", name="psz", bufs=2)
        for h in range(2):
            nc.tensor.matmul(
                psz[:, h * HALF:(h + 1) * HALF], lhsT=zcol,
                rhs=zrow[:, 0:HALF], start=True, stop=False,
                skip_group_check=True)
        segs = []            # (m, k, lo, hi) with [lo,hi) within one half
        for m in range(FM):
            for k in range(NTT):
                a0 = P * k
                cols = min(FW, L - a0)
                cuts = [a0, a0 + cols]
                if a0 < HALF < a0 + cols:
                    cuts = [a0, HALF, a0 + cols]
                for lo, hi in zip(cuts[:-1], cuts[1:]):
                    segs.append((m, k, lo, hi))
        last = {}
        for i, (m, k, lo, hi) in enumerate(segs):
            last[lo // HALF] = i
        for i, (m, k, lo, hi) in enumerate(segs):
            a0 = P * k
            nc.tensor.matmul(
                psz[:, lo:hi],
                lhsT=vT[:, k, D * P:(D + 1) * P],
                rhs=Ks[(m, k)][:, lo - a0:hi - a0],
                start=False, stop=(last[lo // HALF] == i),
                skip_group_check=True)
        pend.append((D, psz, xcr2, szr))
        if len(pend) > 1:
            emit_gate(*pend.pop(0))
    while pend:
        emit_gate(*pend.pop(0))

    # ---------------- out-projection (h-half PSUMs) ---------------------
    for mt in range(NKT):
        wout = tmp.tile([P, NDT, P], BF, tag="wout", name="wout", bufs=2)
        nc.sync.dma_start(wout, dr[f"wout_{sfx}"][:, :, mt * P:(mt + 1) * P])
        for h in range(2):
            ps = psKp.tile([P, HALF], F32, tag="psy", name="psydm", bufs=2)
            for kt in range(NDT):
                nc.tensor.matmul(
                    ps, lhsT=wout[:, kt, :],
                    rhs=yg_sb[kt][:, h * HALF:(h + 1) * HALF],
                    start=(kt == 0), stop=(kt == NDT - 1))
            t = tmp.tile([P, HALF], F32, tag="ydmout", name="ydmout", bufs=2)
            nc.scalar.activation(t, ps, AF.Copy)
            nc.sync.dma_start(
                dr[f"ydmscratch_{sfx}"][mt][:, h * HALF:(h + 1) * HALF], t)


def _build(ln_trivial=False, fast=False):
    """Build + compile the per-core Bass program (identical on all cores)."""
    import concourse.bass as bass  # noqa: F401
    import concourse.tile as tile
    from concourse import bacc, mybir

    AL = mybir.AluOpType
    AF = mybir.ActivationFunctionType
    F32, BF = mybir.dt.float32, mybir.dt.bfloat16

    nc = bacc.Bacc("TRN2", target_bir_lowering=False, debug=False,
                   num_devices=8)

    dr = {}

    def din(name, shape, dt):
        dr[name] = nc.dram_tensor(name, shape, dt, kind="ExternalInput").ap()

    din("xT", [P, NKT, L], BF)
    din("xrevT", [P, NKT, L], BF)
    din("xtok", [P, NTT, DM], F32)
    if fast:
        din("maskdiag", [P, P], BF)
        for s in ("f", "b"):
            din(f"tpos_{s}", [FM, NDT, N, FW], BF)
            din(f"tneg_{s}", [FM, NDT, N, P], BF)
            din(f"convsc_{s}", [P, NDT, KC], F32)
    for s in ("f", "b"):
        din(f"win_{s}", [P, NKT, 2 * DI], BF)
        din(f"convdiag_{s}", [P, NDT * KC * P], BF)
        din(f"convb_{s}", [P, NDT], F32)
        din(f"wx_{s}", [P, NDT, R + 2 * N], BF)
        din(f"wdt_{s}", [R, DI], BF)
        din(f"dtb_{s}", [P, NDT], F32)
        din(f"aneg_{s}", [P, NDT, N], F32)
        din(f"dvec_{s}", [P, NDT], F32)
        din(f"wout_{s}", [P, NDT, DM], BF)
        dr[f"szscratch_{s}"] = nc.dram_tensor(
            f"szscratch_{s}", [NDT, P, L], BF, kind="Internal").ap()
        dr[f"xcscratch_{s}"] = nc.dram_tensor(
            f"xcscratch_{s}", [NDT, P, L], BF, kind="Internal").ap()
        dr[f"dbcscratch_{s}"] = nc.dram_tensor(
            f"dbcscratch_{s}", [R + 2 * N, L], BF, kind="Internal").ap()
        dr[f"ydmscratch_{s}"] = nc.dram_tensor(
            f"ydmscratch_{s}", [NKT, P, L], mybir.dt.float32,
            kind="Internal").ap()
    din("lng", [1, DM], F32)
    din("lnb", [1, DM], F32)
    din("ident32", [P, P], F32)
    din("identbf", [P, P], BF)
    din("jmat", [P, P], F32)
    out_d = nc.dram_tensor("out", [L, DM], F32, kind="ExternalOutput").ap()

    with tile.TileContext(nc) as tc, ExitStack() as octx:
        consts = octx.enter_context(tc.tile_pool(name="consts", bufs=1))
        i32 = consts.tile([P, P], F32)
        nc.sync.dma_start(i32, dr["ident32"])
        ibf = consts.tile([P, P], BF)
        nc.sync.dma_start(ibf, dr["identbf"])
        jm = consts.tile([P, P], F32)
        nc.sync.dma_start(jm, dr["jmat"])
        gbc = consts.tile([P, DM], F32)
        lng = dr["lng"]
        nc.gpsimd.dma_start(out=gbc, in_=bass.AP(
            tensor=lng.tensor, offset=lng.offset,
            ap=[[0, P]] + lng.ap[1:]))
        bbc = consts.tile([P, DM], F32)
        lnb = dr["lnb"]
        nc.gpsimd.dma_start(out=bbc, in_=bass.AP(
            tensor=lnb.tensor, offset=lnb.offset,
            ap=[[0, P]] + lnb.ap[1:]))
        epst = consts.tile([P, 1], F32)
        nc.vector.memset(epst, 1e-5)
        onep = consts.tile([P, 1], F32)
        nc.vector.memset(onep, 1.0)
        if fast:
            maskd = consts.tile([P, P], BF)
            nc.sync.dma_start(maskd, dr["maskdiag"])
            zcol = consts.tile([1, P], BF)
            nc.vector.memset(zcol, 0.0)
            zrow = consts.tile([1, HALF], BF)
            nc.vector.memset(zrow, 0.0)

        # Emission order: A_f -> A_b -> S_f -> S_b.  Stage-A of dir b
        # executes under the forward scans; each engine's in-order queue
        # always has ready work at the phase boundary.  Pool open/close
        # is strictly LIFO (Tile requirement).
        with ExitStack() as dctx_f, ExitStack() as dctx_b:
            st_f = _emit_stage_a(nc, tc, dctx_f, dr, "f", "xT", fast=fast)
            _emit_dt_phase(nc, tc, dctx_f, dr, "f", st_f, onep)
            st_b = _emit_stage_a(nc, tc, dctx_b, dr, "b", "xrevT",
                                 fast=fast)
            _emit_dt_phase(nc, tc, dctx_b, dr, "b", st_b, onep)
            if fast:
                for sfx, stx in (("f", st_f), ("b", st_b)):
                    with ExitStack() as sctx:
                        _emit_fast_ssm(nc, tc, sctx, dr, sfx, stx, ibf,
                                       maskd, zcol, zrow)
            else:
                for sfx, stx in (("f", st_f), ("b", st_b)):
                    with ExitStack() as rctx:
                        _emit_reps(nc, tc, rctx, dr, sfx, stx)
                        with ExitStack() as sctx:
                            _emit_scan(nc, tc, sctx, dr, sfx, stx, ibf,
                                       onep)

        # =================== combine + LayerNorm ===================
        with ExitStack() as cctx:
            cpool = cctx.enter_context(tc.tile_pool(name="comb", bufs=2))
            spool = cctx.enter_context(tc.tile_pool(name="stats", bufs=3))
            psC = cctx.enter_context(
                tc.tile_pool(name="psC", bufs=3, space="PSUM"))
            psT = cctx.enter_context(
                tc.tile_pool(name="psT", bufs=4, space="PSUM"))
            CDT = F32
            idq = i32
            jmq = jm
            xtok = cpool.tile([P, NTT, DM], CDT, tag="xtok", bufs=1)
            nc.sync.dma_start(xtok, dr["xtok"])
            ydm = {}
            for sfx in ("f", "b"):
                ydm[sfx] = []
                for mt in range(NKT):
                    t = cpool.tile([P, L], F32, tag=f"ydm_{sfx}{mt}",
                                   name=f"ydm_{sfx}{mt}", bufs=1)
                    nc.sync.dma_start(t, dr[f"ydmscratch_{sfx}"][mt])
                    ydm[sfx].append(t)
            for tt in range(NTT):
                # transpose both directions' dm-major tiles to token-major
                yft = cpool.tile([P, DM], CDT, tag="yft")
                ybr = cpool.tile([P, DM], CDT, tag="ybr")
                for mt in range(NKT):
                    tp = psT.tile([P, P], CDT, tag="psT")
                    nc.tensor.transpose(
                        tp, ydm["f"][mt][:, tt * P:(tt + 1) * P], idq)
                    nc.scalar.activation(
                        yft[:, mt * P:(mt + 1) * P], tp, AF.Copy)
                    tp2 = psT.tile([P, P], CDT, tag="psT")
                    nc.tensor.transpose(
                        tp2, ydm["b"][mt][:, (NTT - 1 - tt) * P:
                                          (NTT - tt) * P], idq)
                    nc.scalar.activation(
                        ybr[:, mt * P:(mt + 1) * P], tp2, AF.Copy)
                # ytot = x + y_fwd + J @ y_bwd_rev.  Only the J row-reversal
                # needs the PE; the two adds run on the DVE (fp32 matmuls
                # cost 4 cyc/row, the PE is the critical engine).
                yb = psC.tile([P, DM], F32, tag="psC")
                nc.tensor.matmul(yb, lhsT=jmq, rhs=ybr,
                                 start=True, stop=True)
                ys1 = cpool.tile([P, DM], F32, tag="ys1")
                nc.vector.tensor_tensor(ys1, xtok[:, tt, :], yft, AL.add)
                yt = cpool.tile([P, DM], F32, tag="ysum")
                nc.vector.tensor_tensor(yt, ys1, yb, AL.add)
                # LayerNorm over DM (free dim, fp32)
                stats = spool.tile([P, 6], F32, tag="bn")
                nc.vector.bn_stats(stats, yt)
                mv = spool.tile([P, 2], F32, tag="mv")
                nc.vector.bn_aggr(mv, stats)
                sd = spool.tile([P, 1], F32, tag="sd")
                nc.scalar.activation(sd, mv[:, 1:2], AF.Sqrt, bias=epst)
                rs = spool.tile([P, 1], F32, tag="rs")
                nc.vector.reciprocal(rs, sd)
                nmu = spool.tile([P, 1], F32, tag="nmu")
                nc.vector.scalar_tensor_tensor(
                    nmu, mv[:, 0:1], -1.0, rs, AL.mult, AL.mult)
                ycn = cpool.tile([P, DM], F32, tag="ycn")
                nc.scalar.activation(ycn, yt, AF.Identity,
                                     bias=nmu, scale=rs)
                if ln_trivial:
                    nc.sync.dma_start(out_d[tt * P:(tt + 1) * P, :], ycn)
                else:
                    o1 = cpool.tile([P, DM], F32, tag="o1")
                    nc.vector.tensor_tensor(o1, ycn, gbc, AL.mult)
                    o2 = cpool.tile([P, DM], F32, tag="o2")
                    nc.vector.tensor_tensor(o2, o1, bbc, AL.add)
                    nc.sync.dma_start(out_d[tt * P:(tt + 1) * P, :], o2)

    nc.compile()
    return nc


def _host_inputs(inputs, perms=None):
    """Shared (per-core-independent) input arrays, SBUF-layouted.
    perms (fast path): per-direction channel permutation applied to every
    d_inner-indexed tensor; out_w rows are permuted too so the output is
    unchanged."""
    bf = ml_dtypes.bfloat16
    f32 = np.float32

    def tile3(a, nk):
        # [nk*P, F] -> [P, nk, F]
        F = a.shape[-1]
        return np.ascontiguousarray(
            a.reshape(nk, P, F).transpose(1, 0, 2))

    inputs = dict(inputs)
    if perms is not None:
        for s in ("f", "b"):
            pm = perms[s]
            w = inputs[f"in_w_{s}"]
            inputs[f"in_w_{s}"] = np.concatenate(
                [w[:, :DI][:, pm], w[:, DI:][:, pm]], axis=1)
            for nm in ("conv_w", "conv_b", "xproj_w", "dt_b", "A_log", "D",
                       "out_w"):
                inputs[f"{nm}_{s}"] = inputs[f"{nm}_{s}"][pm]
            inputs[f"dt_w_{s}"] = inputs[f"dt_w_{s}"][:, pm]

    m = {}
    for s in ("f", "b"):
        m[f"win_{s}"] = tile3(inputs[f"in_w_{s}"], NKT).astype(bf)
        cw = inputs[f"conv_w_{s}"].reshape(NDT, P, KC)
        cd = np.zeros((NDT, KC, P, P), dtype=np.float32)
        for dt_ in range(NDT):
            for k in range(KC):
                np.fill_diagonal(cd[dt_, k], cw[dt_, :, k])
        # lhsT layout: [p, (dt,k)*P + m] with diag on (p == m)
        m[f"convdiag_{s}"] = np.ascontiguousarray(
            cd.transpose(2, 0, 1, 3).reshape(P, NDT * KC * P)).astype(bf)
        m[f"convb_{s}"] = np.ascontiguousarray(
            inputs[f"conv_b_{s}"].reshape(NDT, P).T).astype(f32)
        m[f"wx_{s}"] = tile3(inputs[f"xproj_w_{s}"], NDT).astype(bf)
        m[f"wdt_{s}"] = inputs[f"dt_w_{s}"].astype(bf)
        m[f"dtb_{s}"] = np.ascontiguousarray(
            inputs[f"dt_b_{s}"].reshape(NDT, P).T).astype(f32)
        m[f"aneg_{s}"] = tile3(-np.exp(inputs[f"A_log_{s}"]), NDT).astype(f32)
        m[f"dvec_{s}"] = np.ascontiguousarray(
            inputs[f"D_{s}"].reshape(NDT, P).T).astype(f32)
        m[f"wout_{s}"] = tile3(inputs[f"out_w_{s}"], NDT).astype(bf)
    m["lng"] = inputs["ln_g"].reshape(1, DM).astype(f32)
    m["lnb"] = inputs["ln_b"].reshape(1, DM).astype(f32)
    m["ident32"] = np.eye(P, dtype=f32)
    m["identbf"] = np.eye(P).astype(bf)
    m["jmat"] = np.eye(P, dtype=f32)[::-1].copy()
    return m


def _run(inputs, trace=False, trace_kwargs=None):
    from concourse.bass_utils import run_bass_kernel_spmd

    ln_trivial = bool(
        np.all(np.asarray(inputs["ln_g"]) == 1.0)
        and np.all(np.asarray(inputs["ln_b"]) == 0.0))
    npin = {k: np.asarray(v) for k, v in inputs.items()}
    prep = _fast_prep(npin)
    fast = prep is not None
    key = ("nc", ln_trivial, fast)
    if key not in _CACHE:
        _CACHE[key] = _build(ln_trivial=ln_trivial, fast=fast)
    nc = _CACHE[key]

    bf = ml_dtypes.bfloat16
    x = npin["x"].astype(np.float32)                       # [8, L, DM]
    perms = {s: prep[f"perm_{s}"] for s in ("f", "b")} if fast else None
    shared = _host_inputs({k: v for k, v in npin.items() if k != "x"},
                          perms=perms)
    if fast:
        for s in ("f", "b"):
            shared[f"tpos_{s}"] = prep[f"tpos_{s}"]
            shared[f"tneg_{s}"] = prep[f"tneg_{s}"]
            cw = npin[f"conv_w_{s}"][perms[s]].reshape(NDT, P, KC)
            shared[f"convsc_{s}"] = np.ascontiguousarray(
                cw.transpose(1, 0, 2)).astype(np.float32)
        tau = np.arange(P)
        shared["maskdiag"] = (tau[None, :] >= tau[:, None]).astype(bf)

    in_maps = []
    for c in range(8):
        xb = x[c]                                          # [L, DM]
        m = dict(shared)
        m["xT"] = np.ascontiguousarray(
            xb.T.reshape(NKT, P, L).transpose(1, 0, 2)).astype(bf)
        m["xrevT"] = np.ascontiguousarray(
            xb[::-1].T.reshape(NKT, P, L).transpose(1, 0, 2)).astype(bf)
        m["xtok"] = np.ascontiguousarray(
            xb.reshape(NTT, P, DM).transpose(1, 0, 2)).astype(np.float32)
        in_maps.append(m)

    res = run_bass_kernel_spmd(nc, in_maps, core_ids=list(range(8)),
                               trace=trace, **(trace_kwargs or {}))
    out = np.stack([res.results[c]["out"] for c in range(8)], axis=0)
    return out.astype(np.float32), res


def kernel(**inputs):
    out, _ = _run(inputs)
    return out


if __name__ == "__main__":
    rng = np.random.default_rng(0)
    fake = {"x": rng.standard_normal((8, L, DM), dtype=np.float32)}
    for s in ("f", "b"):
        fake[f"in_w_{s}"] = rng.standard_normal((DM, 2 * DI), dtype=np.float32) * 0.02
        fake[f"conv_w_{s}"] = rng.standard_normal((DI, KC), dtype=np.float32) * 0.3
        fake[f"conv_b_{s}"] = np.zeros(DI, np.float32)
        fake[f"xproj_w_{s}"] = rng.standard_normal((DI, R + 2 * N), dtype=np.float32) * 0.02
        fake[f"dt_w_{s}"] = rng.standard_normal((R, DI), dtype=np.float32) * 0.02
        fake[f"dt_b_{s}"] = rng.standard_normal(DI, dtype=np.float32) * 0.1 - 4.0
        fake[f"A_log_{s}"] = np.tile(np.log(np.arange(1, N + 1, dtype=np.float32)), (DI, 1))
        fake[f"D_{s}"] = np.ones(DI, np.float32)
        fake[f"out_w_{s}"] = rng.standard_normal((DI, DM), dtype=np.float32) * 0.02
    fake["ln_g"] = np.ones(DM, np.float32)
    fake["ln_b"] = np.zeros(DM, np.float32)
    o = kernel(**fake)
    print("out", o.shape, o.dtype, float(np.abs(o).max()))



# revision 52
# speedup vs baseline: 1.1037x; 1.1037x over previous
"""BiMamba block (bidirectional Mamba-1 + residual + LayerNorm) on 8 TRN2
NeuronCores.

Sharding: data-parallel over batch (B=8 -> one batch element per core).
Each core runs both directions for its batch element; no collectives.

Two compiled paths, selected at run time by an exact host-side check
(_fast_prep):

FAST PATH (banded low-rank kernel; used when A_log rows are identical
and dt is near-constant over time per channel -- both verified on the
host against the actual inputs, with exact-dt recomputation):
  The selective scan y[d,t] = sum_n C[n,t] h[d,n,t] collapses to
  y[d,t] = sum_{s<=t} K_d[t,s] v[d,s] with a rank-N kernel
  K_d[t,s] = sum_n B[n,s] C[n,t] e^{A_n u_d (t-s)} that depends on the
  channel only through the scalar mean step u_d = softplus(dt_b[d]).
  Channels are sorted by u_d on the host (weights permuted; out_w rows
  permuted back), so each 128-channel group shares one kernel built at
  the group's Chebyshev midpoint node (FM=1; rel interp error ~1e-6,
  far below bf16 noise).  The kernel is banded (decay cutoff, window
  FW=384 per 128-row s-tile) and built on the PE as 16-contraction
  matmuls from table-scaled B/C windows; z accumulates banded window
  matmuls into PSUM [128ch, L].  The DVE scan, the exp(dt*A) ACT
  volume, and the w/hC elementwise volumes all disappear; the whole
  SSM becomes PE matmul work.  Measured ~2.4e-4 rel err, ~451 us on HW
  (2.6x over the v2 scan kernel).
  Engine split: PE in-proj/dbc/dt/vT-transposes/K-build/z/out-proj/
  combine; DVE depthwise conv (per-partition tap scalars), v=dt*xc,
  table scalings, K diagonal causal masks, gates, LN stats; ACT silu/
  softplus/PSUM evictions; GPSIMD idle (it shares the DVE SBUF port --
  offloading there slows the DVE).

FALLBACK (exact selective scan, any inputs): the v2 design -- DVE
tensor_tensor_scan over [P, 4 states x L] streams, ACT exp volume, PE
identity-accumulate state reduction, with hoisted dt chains and
deferred gates.

Backward direction runs the same causal pipeline on the host-reversed
sequence; the combine stage un-reverses it with an anti-identity
matmul, then residual + LayerNorm in fp32 (PE transposes to
token-major, bn_stats/bn_aggr), output [L, DM] per core.
"""

import numpy as np
import ml_dtypes
from contextlib import ExitStack

L, DM, DI, N, R, KC = 1024, 512, 1024, 16, 32, 4
P = 128
HALF = 512          # matmul moving-operand / PSUM-bank chunk (fp32 out)
NDT = DI // P       # 8 d-tiles
NTT = L // P        # 8 token tiles
NKT = DM // P       # 4 dm k-tiles

# fast (banded low-rank kernel) path constants
FM = 1              # Chebyshev nodes per sorted channel group
FW = 384            # K window width per s-tile (band 256..383)

_CACHE = {}


def _np_softplus(x):
    return np.log1p(np.exp(-np.abs(x))) + np.maximum(x, 0)


def _np_silu(x):
    return x / (1 + np.exp(-x))


def _fast_prep(inputs):
    """Host gate + per-direction tables for the fast path.

    The fast path replaces the sequential selective scan with banded
    low-rank kernel matmuls.  Valid when (a) A_log rows are identical
    (decay rates shared across channels) and (b) dt is nearly constant
    in time per channel (selectivity small), both checked here exactly.
    Returns None if the inputs don't qualify (kernel falls back to the
    exact scan path)."""
    x = np.asarray(inputs["x"], np.float32)
    prep = {}
    for s in ("f", "b"):
        al = np.asarray(inputs[f"A_log_{s}"])
        if al.shape != (DI, N) or np.ptp(al, axis=0).max() != 0.0:
            return None
        xs = x if s == "f" else x[:, ::-1]
        w_in = np.asarray(inputs[f"in_w_{s}"], np.float32)
        xi = xs.reshape(-1, DM) @ w_in[:, :DI]
        xi = xi.reshape(8, L, DI)
        xp = np.pad(xi, ((0, 0), (KC - 1, 0), (0, 0)))
        cw = np.asarray(inputs[f"conv_w_{s}"], np.float32)
        xc = sum(xp[:, k:k + L, :] * cw[:, k] for k in range(KC))
        xc = _np_silu(xc + np.asarray(inputs[f"conv_b_{s}"], np.float32))
        wx = np.asarray(inputs[f"xproj_w_{s}"], np.float32)
        dtarg = (xc.reshape(-1, DI) @ wx[:, :R]) @ \
            np.asarray(inputs[f"dt_w_{s}"], np.float32)
        dt = _np_softplus(dtarg.reshape(8, L, DI)
                          + np.asarray(inputs[f"dt_b_{s}"], np.float32))
        dbar = _np_softplus(np.asarray(inputs[f"dt_b_{s}"], np.float32))
        if np.abs(dt - dbar[None, None, :]).max() > 3e-3:
            return None
        perm = np.argsort(dbar, kind="stable")
        u = dbar[perm]
        A1 = -np.exp(al[0]).astype(np.float64)          # [N], shared
        tpos = np.zeros((FM, NDT, N, FW), np.float64)
        tneg = np.zeros((FM, NDT, N, P), np.float64)
        phi = np.zeros((FM, DI), np.float64)
        tau = np.arange(FW, dtype=np.float64)
        sig = np.arange(P, dtype=np.float64)
        for D in range(NDT):
            ug = u[D * P:(D + 1) * P]
            lo, hi = float(ug.min()), float(ug.max())
            if FM > 1:
                kk = np.arange(FM)
                um = (lo + hi) / 2 + (hi - lo) / 2 * np.cos(
                    (2 * kk + 1) * np.pi / (2 * FM))
            else:
                um = np.array([(lo + hi) / 2])
            for m in range(FM):
                pm = np.ones(P)
                for m2 in range(FM):
                    if m2 != m:
                        pm *= (ug - um[m2]) / (um[m] - um[m2])
                phi[m, D * P:(D + 1) * P] = pm
                tpos[m, D] = np.exp(A1[:, None] * um[m] * tau[None, :])
                tneg[m, D] = np.exp(-A1[:, None] * um[m] * sig[None, :])
        bf = ml_dtypes.bfloat16
        prep[f"perm_{s}"] = perm
        prep[f"tpos_{s}"] = tpos.astype(bf)
        prep[f"tneg_{s}"] = tneg.astype(bf)
        prep[f"phi_{s}"] = phi.astype(bf)
    return prep


def _emit_stage_a(nc, tc, actx, dr, sfx, xin_name, fast=False):
    """Projections + conv for one direction. Returns the tensors the scan
    stage needs (persistent pool `pool` lives until the scan is done).
    fast=True: depthwise conv runs on the DVE (tensor_scalar + STT chain
    with per-partition tap weights) instead of PE diag matmuls."""
    from concourse import mybir
    AL = mybir.AluOpType
    AF = mybir.ActivationFunctionType
    F32, BF = mybir.dt.float32, mybir.dt.bfloat16

    pool = actx.enter_context(tc.tile_pool(name=f"dir_{sfx}", bufs=1))
    st = {"pool": pool}

    convb = pool.tile([P, NDT], F32, name=f"convb_{sfx}")
    nc.sync.dma_start(convb, dr[f"convb_{sfx}"])
    dtb = pool.tile([P, NDT], F32, name=f"dtb_{sfx}")
    nc.sync.dma_start(dtb, dr[f"dtb_{sfx}"])
    aneg = pool.tile([P, NDT, N], F32, name=f"aneg_{sfx}")
    nc.sync.dma_start(aneg, dr[f"aneg_{sfx}"])
    dvec = pool.tile([P, NDT], F32, name=f"dvec_{sfx}")
    nc.sync.dma_start(dvec, dr[f"dvec_{sfx}"])
    st["aneg"], st["dvec"] = aneg, dvec

    sz_dram = dr[f"szscratch_{sfx}"]
    xc_dram = dr[f"xcscratch_{sfx}"]

    wdt = pool.tile([R, DI], BF, name=f"wdt_{sfx}")
    nc.sync.dma_start(wdt, dr[f"wdt_{sfx}"])
    st["wdt"] = wdt

    with ExitStack() as sctx:
        apool = sctx.enter_context(tc.tile_pool(name=f"stgA_{sfx}", bufs=1))
        atmp = sctx.enter_context(tc.tile_pool(name=f"stgAt_{sfx}", bufs=3))
        psA = sctx.enter_context(
            tc.tile_pool(name=f"psA_{sfx}", bufs=2, space="PSUM"))

        if fast:
            convsc = apool.tile([P, NDT, KC], F32, name=f"convsc_{sfx}")
            nc.sync.dma_start(convsc, dr[f"convsc_{sfx}"])
        else:
            convdiag = apool.tile([P, NDT * KC * P], BF,
                                  name=f"convdiag_{sfx}")
            nc.sync.dma_start(convdiag, dr[f"convdiag_{sfx}"])
        wx = apool.tile([P, NDT, R + 2 * N], BF, name=f"wx_{sfx}")
        nc.sync.dma_start(wx, dr[f"wx_{sfx}"])
        xT = apool.tile([P, NKT, L], BF, name=f"xT_{sfx}")
        nc.sync.dma_start(xT, dr[xin_name])

        # in-projection xz = x @ Win, xi and z halves interleaved per
        # channel so the PE has ~6us of matmul work per channel while the
        # DVE conv chain (~3us) trails without stalling anything.
        xc_sb = []
        winh = apool.tile([P, NKT, DI], BF, tag="win1", name=f"win_{sfx}")
        nc.sync.dma_start(winh, dr[f"win_{sfx}"][:, :, 0:DI])
        winh2 = apool.tile([P, NKT, DI], BF, tag="win2", name=f"win2_{sfx}")
        for ch in range(NDT):
            ps = psA.tile([P, L], F32, tag="psA", name="psxz")
            for h in range(2):
                for kt in range(NKT):
                    nc.tensor.matmul(
                        ps[:, h * HALF:(h + 1) * HALF],
                        lhsT=winh[:, kt, ch * P:(ch + 1) * P],
                        rhs=xT[:, kt, h * HALF:(h + 1) * HALF],
                        start=(kt == 0), stop=(kt == NKT - 1))
            if ch == 0:
                # defer the z-half weight DMA so it never delays the
                # first xi matmuls (it's only needed at z(0), ~6us later)
                nc.sync.dma_start(winh2, dr[f"win_{sfx}"][:, :, DI:2 * DI])
            xi = atmp.tile([P, L + KC - 1], BF, tag="xi", name="xi")
            nc.vector.memset(xi[:, 0:KC - 1], 0.0)
            nc.scalar.activation(xi[:, KC - 1:], ps, AF.Copy)
            if fast:
                # depthwise causal conv on DVE: per-tap tensor_scalar
                # (4x mode) + a 2x tensor_tensor add tree.
                tks = []
                for k in range(KC):
                    tk = atmp.tile([P, L], BF, tag=f"cvt{k}", name="cvt",
                                   bufs=2)
                    nc.vector.tensor_scalar_mul(
                        tk, xi[:, k:k + L], convsc[:, ch, k:k + 1])
                    tks.append(tk)
                s01 = atmp.tile([P, L], BF, tag="cvs0", name="cvs0", bufs=2)
                nc.vector.tensor_tensor(s01, tks[0], tks[1], AL.add)
                s23 = atmp.tile([P, L], BF, tag="cvs1", name="cvs1", bufs=2)
                nc.vector.tensor_tensor(s23, tks[2], tks[3], AL.add)
                acc = atmp.tile([P, L], BF, tag="cvs2", name="cvs2", bufs=2)
                nc.vector.tensor_tensor(acc, s01, s23, AL.add)
                t = apool.tile([P, L], BF, tag=f"xc{ch}", name=f"xc{ch}")
                nc.scalar.activation(t, acc, AF.Silu,
                                     bias=convb[:, ch:ch + 1])
            else:
                # depthwise causal conv (4 taps) on the PE via diagonal
                # tap matrices over shifted windows.
                cps = psA.tile([P, L], F32, tag="psA", name="pscv")
                for h in range(2):
                    for k in range(KC):
                        nc.tensor.matmul(
                            cps[:, h * HALF:(h + 1) * HALF],
                            lhsT=convdiag[:, (ch * KC + k) * P:
                                          (ch * KC + k + 1) * P],
                            rhs=xi[:, k + h * HALF:k + (h + 1) * HALF],
                            start=(k == 0), stop=(k == KC - 1))
                t = apool.tile([P, L], BF, tag=f"xc{ch}", name=f"xc{ch}")
                nc.scalar.activation(t, cps, AF.Silu,
                                     bias=convb[:, ch:ch + 1])
            xc_sb.append(t)
            ps2 = psA.tile([P, L], F32, tag="psA", name="psz")
            for h in range(2):
                for kt in range(NKT):
                    nc.tensor.matmul(
                        ps2[:, h * HALF:(h + 1) * HALF],
                        lhsT=winh2[:, kt, ch * P:(ch + 1) * P],
                        rhs=xT[:, kt, h * HALF:(h + 1) * HALF],
                        start=(kt == 0), stop=(kt == NKT - 1))
            tz = atmp.tile([P, L], BF, tag="sz", name="sz")
            nc.scalar.activation(tz, ps2, AF.Silu)
            nc.sync.dma_start(sz_dram[ch], tz)

        # x-projection: dbc = xc @ Wx  [R+2N, L] channel-major; keep the
        # SBUF copy persistent (dt is re-derived from it per scan block)
        # and park a DRAM copy for the B/C broadcast DMAs.
        dbc_ps = psA.tile([R + 2 * N, L], F32, tag="psA", name="psdbc")
        for h in range(2):
            for kt in range(NDT):
                nc.tensor.matmul(
                    dbc_ps[:, h * HALF:(h + 1) * HALF],
                    lhsT=wx[:, kt, :],
                    rhs=xc_sb[kt][:, h * HALF:(h + 1) * HALF],
                    start=(kt == 0), stop=(kt == NDT - 1))
        dbc = pool.tile([R + 2 * N, L], BF, name=f"dbc_{sfx}")
        nc.scalar.activation(dbc, dbc_ps, AF.Copy)
        nc.sync.dma_start(dr[f"dbcscratch_{sfx}"], dbc)
        st["dbc"] = dbc

        # park xc to DRAM (re-read at scan time for v = dt*xc + evict)
        for d in range(NDT):
            nc.sync.dma_start(xc_dram[d], xc_sb[d])

    st["dtb"] = dtb
    return st


def _emit_reps(nc, tc, rctx, dr, sfx, st):
    """B/C broadcast super-tiles [P, 4, L]: 4 states per tile via DMA
    partition-broadcast from the DRAM copy of dbc."""
    from concourse import mybir
    import concourse.bass as bass
    BF = mybir.dt.bfloat16

    rpool = rctx.enter_context(tc.tile_pool(name=f"reps_{sfx}", bufs=1))
    brep4, crep4 = [], []
    dbcd = dr[f"dbcscratch_{sfx}"]
    for gi, lst in ((0, brep4), (1, crep4)):
        for g in range(N // 4):
            t = rpool.tile([P, 4, L], BF, name=f"rep{gi}_{g}")
            for j in range(4):
                row = dbcd[R + gi * N + g * 4 + j:
                           R + gi * N + g * 4 + j + 1, :]
                nc.sync.dma_start(out=t[:, j, :], in_=bass.AP(
                    tensor=row.tensor, offset=row.offset,
                    ap=[[0, P]] + row.ap[1:]))
            lst.append(t)
    st.update(brep4=brep4, crep4=crep4)


def _emit_dt_phase(nc, tc, dctx, dr, sfx, st, onep):
    """Hoisted dt computation for all 8 d-tiles of one direction: the PE
    matmuls + ACT softplus chains run ahead of the other direction's
    stage A, so the scans never wait on a cold PE queue.  Also groups all
    Exp/Ln ACT ops (one act-table load per direction)."""
    from concourse import mybir
    AF = mybir.ActivationFunctionType
    F32, BF = mybir.dt.float32, mybir.dt.bfloat16

    wdt, dbc, dtb = st["wdt"], st["dbc"], st["dtb"]
    dtpool = dctx.enter_context(tc.tile_pool(name=f"dtph_{sfx}", bufs=1))
    dts = []
    with ExitStack() as pctx:
        psD = pctx.enter_context(
            tc.tile_pool(name=f"psD_{sfx}", bufs=2, space="PSUM"))
        for d in range(NDT):
            dtps = psD.tile([P, L], F32, tag="psdt", name="psdt")
            for h in range(2):
                nc.tensor.matmul(
                    dtps[:, h * HALF:(h + 1) * HALF],
                    lhsT=wdt[:, d * P:(d + 1) * P],
                    rhs=dbc[0:R, h * HALF:(h + 1) * HALF],
                    start=True, stop=True)
            dtmid = dtpool.tile([P, L], BF, tag="dtm", name="dtm", bufs=2)
            nc.scalar.activation(dtmid, dtps, AF.Exp, bias=dtb[:, d:d + 1])
            dt_d = dtpool.tile([P, L], BF, name=f"dtd{d}")
            nc.scalar.activation(dt_d, dtmid, AF.Ln, bias=onep)
            dts.append(dt_d)
    st["dts"] = dts


def _emit_scan(nc, tc, dctx, dr, sfx, st, ibf, onep):
    """Selective scan + gating + out-projection for one direction.
    All elementwise work on DVE (GPSIMD shares the DVE SBUF port and
    slows the scans); PE accumulates the state-reduce.  Gate ops for
    d-tile k are deferred until after d-tile k+1's scans so the DVE
    never stalls waiting on the PE reduce."""
    from concourse import mybir
    import concourse.bass as bass
    AL = mybir.AluOpType
    AF = mybir.ActivationFunctionType
    F32, BF = mybir.dt.float32, mybir.dt.bfloat16

    brep4, crep4 = st["brep4"], st["crep4"]
    aneg, dvec = st["aneg"], st["dvec"]
    dts, dtb = st["dts"], st["dtb"]

    def rep4(ap2d):
        return bass.AP(tensor=ap2d.tensor, offset=ap2d.offset,
                       ap=[ap2d.ap[0], [0, 4]] + ap2d.ap[1:])

    yg_sb = []
    tmp2 = dctx.enter_context(tc.tile_pool(name=f"tmp_{sfx}", bufs=2))
    scanp = dctx.enter_context(tc.tile_pool(name=f"scan_{sfx}", bufs=3))
    psY = dctx.enter_context(
        tc.tile_pool(name=f"psY_{sfx}", bufs=2, space="PSUM"))

    pend = []   # deferred gate work: (d, yps, xcr, szr)

    def emit_gate(d, yps, xcr, szr):
        yd = tmp2.tile([P, L], BF, tag="yd", name="yd")
        nc.vector.scalar_tensor_tensor(
            yd, xcr, dvec[:, d:d + 1], yps, AL.mult, AL.add)
        t = tmp2.tile([P, L], BF, tag=f"yg{d}", name=f"yg{d}", bufs=1)
        nc.vector.tensor_tensor(t, yd, szr, AL.mult)
        yg_sb.append(t)

    for d in range(NDT):
        dt_d = dts[d]
        xcr = tmp2.tile([P, L], BF, tag="xcr", name="xcr", bufs=2)
        nc.sync.dma_start(xcr, dr[f"xcscratch_{sfx}"][d])
        szr = tmp2.tile([P, L], BF, tag="szr", name="szr", bufs=2)
        nc.sync.dma_start(szr, dr[f"szscratch_{sfx}"][d])
        v_d = tmp2.tile([P, L], BF, tag="vd", name="vd", bufs=1)
        nc.vector.tensor_tensor(v_d, dt_d, xcr, AL.mult)
        yps = psY.tile([P, L], F32, tag="psY", name="psy")
        for g in range(N // 4):
            # w4/h4 at bufs=1 is stall-free: their producer/consumer pairs
            # are adjacent in DVE program order anyway.
            w4 = scanp.tile([P, 4, L], BF, tag="w4", name="w4", bufs=1)
            nc.vector.tensor_tensor(w4, rep4(v_d), brep4[g], AL.mult)
            # a = exp(dt*A_n); a=0 at each state's t=0 resets the carried
            # state exactly (h = 0*h_prev + w[0]).
            a4 = scanp.tile([P, 4, L], BF, tag="a4", name="a4", bufs=2)
            for j in range(4):
                n = g * 4 + j
                nc.scalar.activation(
                    a4[:, j, :], dt_d, AF.Exp,
                    scale=aneg[:, d, n:n + 1])
            nc.vector.memset(a4[:, 1:4, 0:1], 0.0)
            h4 = scanp.tile([P, 4, L], BF, tag="h4", name="h4", bufs=1)
            nc.vector.tensor_tensor_scan(
                h4.rearrange("p a b -> p (a b)"),
                a4.rearrange("p a b -> p (a b)"),
                w4.rearrange("p a b -> p (a b)"), 0.0, AL.mult, AL.add)
            hc4 = scanp.tile([P, 4, L], BF, tag="hc4", name="hc4", bufs=2)
            nc.vector.tensor_tensor(hc4, h4, crep4[g], AL.mult)
            for j in range(4):
                for h in range(2):
                    nc.tensor.matmul(
                        yps[:, h * HALF:(h + 1) * HALF],
                        lhsT=ibf,
                        rhs=hc4[:, j, h * HALF:(h + 1) * HALF],
                        start=(g == 0 and j == 0),
                        stop=(g == N // 4 - 1 and j == 3))
        # evict: yg = (y + xc*D) * silu(z) — deferred one d-tile so the
        # DVE's queue head never blocks on the PE reduce of this d-tile.
        pend.append((d, yps, xcr, szr))
        if len(pend) > 1:
            dd, yy, xx, ss = pend.pop(0)
            emit_gate(dd, yy, xx, ss)
    while pend:
        dd, yy, xx, ss = pend.pop(0)
        emit_gate(dd, yy, xx, ss)

    # out-projection: ydm = yg @ Wout  [DM, L] dm-major, f32 -> DRAM
    for mt in range(NKT):
        wout = tmp2.tile([P, NDT, P], BF, tag="wout", name="wout", bufs=2)
        nc.sync.dma_start(wout, dr[f"wout_{sfx}"][:, :, mt * P:(mt + 1) * P])
        ps = psY.tile([P, L], F32, tag="psY", name="psydm")
        for h in range(2):
            for kt in range(NDT):
                nc.tensor.matmul(
                    ps[:, h * HALF:(h + 1) * HALF],
                    lhsT=wout[:, kt, :],
                    rhs=yg_sb[kt][:, h * HALF:(h + 1) * HALF],
                    start=(kt == 0), stop=(kt == NDT - 1))
        t = tmp2.tile([P, L], F32, tag="ydmout", name="ydmout", bufs=1)
        nc.scalar.activation(t, ps, AF.Copy)
        nc.sync.dma_start(dr[f"ydmscratch_{sfx}"][mt], t)


def _emit_fast_ssm(nc, tc, dctx, dr, sfx, st, ibf, maskd, zcol, zrow):
    """Banded low-rank kernel path for one direction (replaces the scan).

    y[d,t] = sum_{s<=t, t-s<=band} K_D[s,t] * v[d,s] per sorted channel
    group D, where K_D[s,t] = sum_n B[n,s] C[n,t] e^{A_n u_D (t-s)} is a
    rank-N kernel shared by the group's 128 channels (u_D = group-center
    mean dt).  Per s-tile k the kernel occupies a [128, FW] window built
    as one 16-contraction matmul from table-scaled B/C; z accumulates
    window matmuls into PSUM [128d, L]."""
    from concourse import mybir
    import concourse.bass as bass
    AL = mybir.AluOpType
    AF = mybir.ActivationFunctionType
    F32, BF = mybir.dt.float32, mybir.dt.bfloat16

    dts, dvec = st["dts"], st["dvec"]
    dbcd = dr[f"dbcscratch_{sfx}"]
    fpool = dctx.enter_context(tc.tile_pool(name=f"fs_{sfx}", bufs=1))
    tmp = dctx.enter_context(tc.tile_pool(name=f"ft_{sfx}", bufs=2))
    kpool = dctx.enter_context(tc.tile_pool(name=f"fk_{sfx}", bufs=2))

    # ---------------- phase 1: v, then vT via PE transposes -------------
    vT = fpool.tile([P, NTT, DI], BF, name=f"vT_{sfx}")
    with ExitStack() as tctx:
        psT = tctx.enter_context(
            tc.tile_pool(name=f"psT_{sfx}", bufs=2, space="PSUM"))
        for d in range(NDT):
            xcr = tmp.tile([P, L], BF, tag="xcr", name="xcr", bufs=2)
            nc.sync.dma_start(xcr, dr[f"xcscratch_{sfx}"][d])
            v_d = tmp.tile([P, L], BF, tag="vd", name="vd", bufs=2)
            nc.vector.tensor_tensor(v_d, dts[d], xcr, AL.mult)
            for q in range(2):
                tp = psT.tile([P, 4, P], BF, tag="tp", name="tp")
                for j in range(4):
                    k = q * 4 + j
                    nc.tensor.transpose(
                        tp[:, j, :], v_d[:, k * P:(k + 1) * P], ibf)
                nc.scalar.activation(
                    vT[:, q * 4:(q + 1) * 4, d * P:(d + 1) * P], tp, AF.Copy)

    # ------------- phase 2: B/C windows (shared across groups) ----------
    cwin = fpool.tile([N, NTT, FW], BF, name=f"cwin_{sfx}")
    bwin = fpool.tile([N, NTT, P], BF, name=f"bwin_{sfx}")
    for k in range(NTT):
        cols = min(FW, L - P * k)
        nc.sync.dma_start(cwin[:, k, 0:cols],
                          dbcd[R + N:R + 2 * N, P * k:P * k + cols])
        nc.sync.dma_start(bwin[:, k, :],
                          dbcd[R:R + N, P * k:P * k + P])

    # ---------------- phase 3: per-group K build + z + gates ------------
    psKp = dctx.enter_context(
        tc.tile_pool(name=f"psK_{sfx}", bufs=1, space="PSUM"))
    psZp = dctx.enter_context(
        tc.tile_pool(name=f"psZ_{sfx}", bufs=1, space="PSUM"))

    def repk(ap2d, n):
        return bass.AP(tensor=ap2d.tensor, offset=ap2d.offset,
                       ap=[ap2d.ap[0], [0, n]] + ap2d.ap[1:])

    def build_K(D):
        ct, bt = {}, {}
        for m in range(FM):
            tpt = tmp.tile([N, FW], BF, tag=f"tp{m}", name="tpt", bufs=3)
            nc.sync.dma_start(tpt, dr[f"tpos_{sfx}"][m][D])
            tnt = tmp.tile([N, P], BF, tag=f"tn{m}", name="tnt", bufs=3)
            nc.sync.dma_start(tnt, dr[f"tneg_{sfx}"][m][D])
            c = tmp.tile([N, NTT, FW], BF, tag=f"ct{m}", name="ct", bufs=3)
            nc.vector.tensor_tensor(c, cwin, repk(tpt, NTT), AL.mult)
            b = tmp.tile([N, NTT, P], BF, tag=f"bt{m}", name="bt", bufs=3)
            nc.vector.tensor_tensor(b, bwin, repk(tnt, NTT), AL.mult)
            ct[m], bt[m] = c, b
        Ks = {}
        for m in range(FM):
            for k in range(NTT):
                cols = min(FW, L - P * k)
                psk = psKp.tile([P, FW], F32, tag="psk", name="psk", bufs=2)
                nc.tensor.matmul(
                    psk[:, 0:cols],
                    lhsT=bt[m][:, k, :],
                    rhs=ct[m][:, k, 0:cols],
                    start=True, stop=True)
                Kt = kpool.tile([P, FW], BF, tag=f"K{m}_{k}", name="Kt",
                                bufs=3)
                # causal mask on the diagonal 128 cols (DVE), plain copy
                # for the rest (ACT) — splits the evict load.
                nc.vector.tensor_tensor(Kt[:, 0:P], psk[:, 0:P], maskd,
                                        AL.mult)
                if cols > P:
                    nc.scalar.activation(Kt[:, P:cols], psk[:, P:cols],
                                         AF.Copy)
                Ks[(m, k)] = Kt
        return Ks

    yg_sb = []
    pend = []

    def emit_gate(D, psz, xcr2, szr):
        yd = tmp.tile([P, L], BF, tag="yd", name="yd")
        nc.vector.scalar_tensor_tensor(
            yd, xcr2, dvec[:, D:D + 1], psz, AL.mult, AL.add)
        t = tmp.tile([P, L], BF, tag=f"yg{D}", name=f"yg{D}", bufs=1)
        nc.vector.tensor_tensor(t, yd, szr, AL.mult)
        yg_sb.append(t)

    Kq = [build_K(0), build_K(1)]
    for D in range(NDT):
        if D + 2 < NDT:
            Kq.append(build_K(D + 2))
        Ks = Kq.pop(0)
        xcr2 = tmp.tile([P, L], BF, tag="xcr2", name="xcr2", bufs=2)
        nc.sync.dma_start(xcr2, dr[f"xcscratch_{sfx}"][D])
        szr = tmp.tile([P, L], BF, tag="szr", name="szr", bufs=2)
        nc.sync.dma_start(szr, dr[f"szscratch_{sfx}"][D])
        psz = psZp.tile([P, L], F32, tag="psz", name="psz", bufs=2)
        for h in range(2):
            nc.tensor.matmul(
                psz[:, h * HALF:(h + 1) * HALF], lhsT=zcol,
                rhs=zrow[:, 0:HALF], start=True, stop=False,
                skip_group_check=True)
        segs = []            # (m, k, lo, hi) with [lo,hi) within one half
        for m in range(FM):
            for k in range(NTT):
                a0 = P * k
                cols = min(FW, L - a0)
                cuts = [a0, a0 + cols]
                if a0 < HALF < a0 + cols:
                    cuts = [a0, HALF, a0 + cols]
                for lo, hi in zip(cuts[:-1], cuts[1:]):
                    segs.append((m, k, lo, hi))
        last = {}
        for i, (m, k, lo, hi) in enumerate(segs):
            last[lo // HALF] = i
        for i, (m, k, lo, hi) in enumerate(segs):
            a0 = P * k
            nc.tensor.matmul(
                psz[:, lo:hi],
                lhsT=vT[:, k, D * P:(D + 1) * P],
                rhs=Ks[(m, k)][:, lo - a0:hi - a0],
                start=False, stop=(last[lo // HALF] == i),
                skip_group_check=True)
        pend.append((D, psz, xcr2, szr))
        if len(pend) > 1:
            emit_gate(*pend.pop(0))
    while pend:
        emit_gate(*pend.pop(0))

    # ---------------- out-projection (h-half PSUMs) ---------------------
    for mt in range(NKT):
        wout = tmp.tile([P, NDT, P], BF, tag="wout", name="wout", bufs=2)
        nc.sync.dma_start(wout, dr[f"wout_{sfx}"][:, :, mt * P:(mt + 1) * P])
        for h in range(2):
            ps = psKp.tile([P, HALF], F32, tag="psy", name="psydm", bufs=2)
            for kt in range(NDT):
                nc.tensor.matmul(
                    ps, lhsT=wout[:, kt, :],
                    rhs=yg_sb[kt][:, h * HALF:(h + 1) * HALF],
                    start=(kt == 0), stop=(kt == NDT - 1))
            t = tmp.tile([P, HALF], F32, tag="ydmout", name="ydmout", bufs=2)
            nc.scalar.activation(t, ps, AF.Copy)
            nc.sync.dma_start(
                dr[f"ydmscratch_{sfx}"][mt][:, h * HALF:(h + 1) * HALF], t)


def _build(ln_trivial=False, fast=False):
    """Build + compile the per-core Bass program (identical on all cores)."""
    import concourse.bass as bass  # noqa: F401
    import concourse.tile as tile
    from concourse import bacc, mybir

    AL = mybir.AluOpType
    AF = mybir.ActivationFunctionType
    F32, BF = mybir.dt.float32, mybir.dt.bfloat16

    nc = bacc.Bacc("TRN2", target_bir_lowering=False, debug=False,
                   num_devices=8)

    dr = {}

    def din(name, shape, dt):
        dr[name] = nc.dram_tensor(name, shape, dt, kind="ExternalInput").ap()

    din("xT", [P, NKT, L], BF)
    din("xrevT", [P, NKT, L], BF)
    din("xtok", [P, NTT, DM], F32)
    if fast:
        din("maskdiag", [P, P], BF)
        for s in ("f", "b"):
            din(f"tpos_{s}", [FM, NDT, N, FW], BF)
            din(f"tneg_{s}", [FM, NDT, N, P], BF)
            din(f"convsc_{s}", [P, NDT, KC], F32)
    for s in ("f", "b"):
        din(f"win_{s}", [P, NKT, 2 * DI], BF)
        din(f"convdiag_{s}", [P, NDT * KC * P], BF)
        din(f"convb_{s}", [P, NDT], F32)
        din(f"wx_{s}", [P, NDT, R + 2 * N], BF)
        din(f"wdt_{s}", [R, DI], BF)
        din(f"dtb_{s}", [P, NDT], F32)
        din(f"aneg_{s}", [P, NDT, N], F32)
        din(f"dvec_{s}", [P, NDT], F32)
        din(f"wout_{s}", [P, NDT, DM], BF)
        dr[f"szscratch_{s}"] = nc.dram_tensor(
            f"szscratch_{s}", [NDT, P, L], BF, kind="Internal").ap()
        dr[f"xcscratch_{s}"] = nc.dram_tensor(
            f"xcscratch_{s}", [NDT, P, L], BF, kind="Internal").ap()
        dr[f"dbcscratch_{s}"] = nc.dram_tensor(
            f"dbcscratch_{s}", [R + 2 * N, L], BF, kind="Internal").ap()
        dr[f"ydmscratch_{s}"] = nc.dram_tensor(
            f"ydmscratch_{s}", [NKT, P, L], mybir.dt.float32,
            kind="Internal").ap()
    din("lng", [1, DM], F32)
    din("lnb", [1, DM], F32)
    din("ident32", [P, P], F32)
    din("identbf", [P, P], BF)
    din("jmat", [P, P], F32)
    out_d = nc.dram_tensor("out", [L, DM], F32, kind="ExternalOutput").ap()

    with tile.TileContext(nc) as tc, ExitStack() as octx:
        consts = octx.enter_context(tc.tile_pool(name="consts", bufs=1))
        i32 = consts.tile([P, P], F32)
        nc.sync.dma_start(i32, dr["ident32"])
        ibf = consts.tile([P, P], BF)
        nc.sync.dma_start(ibf, dr["identbf"])
        jm = consts.tile([P, P], F32)
        nc.sync.dma_start(jm, dr["jmat"])
        gbc = consts.tile([P, DM], F32)
        lng = dr["lng"]
        nc.gpsimd.dma_start(out=gbc, in_=bass.AP(
            tensor=lng.tensor, offset=lng.offset,
            ap=[[0, P]] + lng.ap[1:]))
        bbc = consts.tile([P, DM], F32)
        lnb = dr["lnb"]
        nc.gpsimd.dma_start(out=bbc, in_=bass.AP(
            tensor=lnb.tensor, offset=lnb.offset,
            ap=[[0, P]] + lnb.ap[1:]))
        epst = consts.tile([P, 1], F32)
        nc.vector.memset(epst, 1e-5)
        onep = consts.tile([P, 1], F32)
        nc.vector.memset(onep, 1.0)
        if fast:
            maskd = consts.tile([P, P], BF)
            nc.sync.dma_start(maskd, dr["maskdiag"])
            zcol = consts.tile([1, P], BF)
            nc.vector.memset(zcol, 0.0)
            zrow = consts.tile([1, HALF], BF)
            nc.vector.memset(zrow, 0.0)

        # Emission order: A_f -> A_b -> S_f -> S_b.  Stage-A of dir b
        # executes under the forward scans; each engine's in-order queue
        # always has ready work at the phase boundary.  Pool open/close
        # is strictly LIFO (Tile requirement).
        with ExitStack() as dctx_f, ExitStack() as dctx_b:
            st_f = _emit_stage_a(nc, tc, dctx_f, dr, "f", "xT", fast=fast)
            _emit_dt_phase(nc, tc, dctx_f, dr, "f", st_f, onep)
            st_b = _emit_stage_a(nc, tc, dctx_b, dr, "b", "xrevT",
                                 fast=fast)
            _emit_dt_phase(nc, tc, dctx_b, dr, "b", st_b, onep)
            if fast:
                for sfx, stx in (("f", st_f), ("b", st_b)):
                    with ExitStack() as sctx:
                        _emit_fast_ssm(nc, tc, sctx, dr, sfx, stx, ibf,
                                       maskd, zcol, zrow)
            else:
                for sfx, stx in (("f", st_f), ("b", st_b)):
                    with ExitStack() as rctx:
                        _emit_reps(nc, tc, rctx, dr, sfx, stx)
                        with ExitStack() as sctx:
                            _emit_scan(nc, tc, sctx, dr, sfx, stx, ibf,
                                       onep)

        # =================== combine + LayerNorm ===================
        with ExitStack() as cctx:
            cpool = cctx.enter_context(tc.tile_pool(name="comb", bufs=2))
            spool = cctx.enter_context(tc.tile_pool(name="stats", bufs=3))
            psC = cctx.enter_context(
                tc.tile_pool(name="psC", bufs=3, space="PSUM"))
            psT = cctx.enter_context(
                tc.tile_pool(name="psT", bufs=4, space="PSUM"))
            CDT = F32
            idq = i32
            jmq = jm
            xtok = cpool.tile([P, NTT, DM], CDT, tag="xtok", bufs=1)
            nc.sync.dma_start(xtok, dr["xtok"])
            ydm = {}
            for sfx in ("f", "b"):
                ydm[sfx] = []
                for mt in range(NKT):
                    t = cpool.tile([P, L], F32, tag=f"ydm_{sfx}{mt}",
                                   name=f"ydm_{sfx}{mt}", bufs=1)
                    nc.sync.dma_start(t, dr[f"ydmscratch_{sfx}"][mt])
                    ydm[sfx].append(t)
            for tt in range(NTT):
                # transpose both directions' dm-major tiles to token-major
                yft = cpool.tile([P, DM], CDT, tag="yft")
                ybr = cpool.tile([P, DM], CDT, tag="ybr")
                for mt in range(NKT):
                    tp = psT.tile([P, P], CDT, tag="psT")
                    nc.tensor.transpose(
                        tp, ydm["f"][mt][:, tt * P:(tt + 1) * P], idq)
                    nc.scalar.activation(
                        yft[:, mt * P:(mt + 1) * P], tp, AF.Copy)
                    tp2 = psT.tile([P, P], CDT, tag="psT")
                    nc.tensor.transpose(
                        tp2, ydm["b"][mt][:, (NTT - 1 - tt) * P:
                                          (NTT - tt) * P], idq)
                    nc.scalar.activation(
                        ybr[:, mt * P:(mt + 1) * P], tp2, AF.Copy)
                # ytot = x + y_fwd + J @ y_bwd_rev.  Only the J row-reversal
                # needs the PE; the two adds run on the DVE (fp32 matmuls
                # cost 4 cyc/row, the PE is the critical engine).
                yb = psC.tile([P, DM], F32, tag="psC")
                nc.tensor.matmul(yb, lhsT=jmq, rhs=ybr,
                                 start=True, stop=True)
                ys1 = cpool.tile([P, DM], F32, tag="ys1")
                nc.vector.tensor_tensor(ys1, xtok[:, tt, :], yft, AL.add)
                yt = cpool.tile([P, DM], F32, tag="ysum")
                nc.vector.tensor_tensor(yt, ys1, yb, AL.add)
                # LayerNorm over DM (free dim, fp32)
                stats = spool.tile([P, 6], F32, tag="bn")
                nc.vector.bn_stats(stats, yt)
                mv = spool.tile([P, 2], F32, tag="mv")
                nc.vector.bn_aggr(mv, stats)
                sd = spool.tile([P, 1], F32, tag="sd")
                nc.scalar.activation(sd, mv[:, 1:2], AF.Sqrt, bias=epst)
                rs = spool.tile([P, 1], F32, tag="rs")
                nc.vector.reciprocal(rs, sd)
                nmu = spool.tile([P, 1], F32, tag="nmu")
                nc.vector.scalar_tensor_tensor(
                    nmu, mv[:, 0:1], -1.0, rs, AL.mult, AL.mult)
                ycn = cpool.tile([P, DM], F32, tag="ycn")
                nc.scalar.activation(ycn, yt, AF.Identity,
                                     bias=nmu, scale=rs)
                if ln_trivial:
                    nc.sync.dma_start(out_d[tt * P:(tt + 1) * P, :], ycn)
                else:
                    o1 = cpool.tile([P, DM], F32, tag="o1")
                    nc.vector.tensor_tensor(o1, ycn, gbc, AL.mult)
                    o2 = cpool.tile([P, DM], F32, tag="o2")
                    nc.vector.tensor_tensor(o2, o1, bbc, AL.add)
                    nc.sync.dma_start(out_d[tt * P:(tt + 1) * P, :], o2)

    nc.compile()
    return nc


def _host_inputs(inputs, perms=None):
    """Shared (per-core-independent) input arrays, SBUF-layouted.
    perms (fast path): per-direction channel permutation applied to every
    d_inner-indexed tensor; out_w rows are permuted too so the output is
    unchanged."""
    bf = ml_dtypes.bfloat16
    f32 = np.float32

    def tile3(a, nk):
        # [nk*P, F] -> [P, nk, F]
        F = a.shape[-1]
        return np.ascontiguousarray(
            a.reshape(nk, P, F).transpose(1, 0, 2))

    inputs = dict(inputs)
    if perms is not None:
        for s in ("f", "b"):
            pm = perms[s]
            w = inputs[f"in_w_{s}"]
            inputs[f"in_w_{s}"] = np.concatenate(
                [w[:, :DI][:, pm], w[:, DI:][:, pm]], axis=1)
            for nm in ("conv_w", "conv_b", "xproj_w", "dt_b", "A_log", "D",
                       "out_w"):
                inputs[f"{nm}_{s}"] = inputs[f"{nm}_{s}"][pm]
            inputs[f"dt_w_{s}"] = inputs[f"dt_w_{s}"][:, pm]

    m = {}
    for s in ("f", "b"):
        m[f"win_{s}"] = tile3(inputs[f"in_w_{s}"], NKT).astype(bf)
        cw = inputs[f"conv_w_{s}"].reshape(NDT, P, KC)
        cd = np.zeros((NDT, KC, P, P), dtype=np.float32)
        for dt_ in range(NDT):
            for k in range(KC):
                np.fill_diagonal(cd[dt_, k], cw[dt_, :, k])
        # lhsT layout: [p, (dt,k)*P + m] with diag on (p == m)
        m[f"convdiag_{s}"] = np.ascontiguousarray(
            cd.transpose(2, 0, 1, 3).reshape(P, NDT * KC * P)).astype(bf)
        m[f"convb_{s}"] = np.ascontiguousarray(
            inputs[f"conv_b_{s}"].reshape(NDT, P).T).astype(f32)
        m[f"wx_{s}"] = tile3(inputs[f"xproj_w_{s}"], NDT).astype(bf)
        m[f"wdt_{s}"] = inputs[f"dt_w_{s}"].astype(bf)
        m[f"dtb_{s}"] = np.ascontiguousarray(
            inputs[f"dt_b_{s}"].reshape(NDT, P).T).astype(f32)
        m[f"aneg_{s}"] = tile3(-np.exp(inputs[f"A_log_{s}"]), NDT).astype(f32)
        m[f"dvec_{s}"] = np.ascontiguousarray(
            inputs[f"D_{s}"].reshape(NDT, P).T).astype(f32)
        m[f"wout_{s}"] = tile3(inputs[f"out_w_{s}"], NDT).astype(bf)
    m["lng"] = inputs["ln_g"].reshape(1, DM).astype(f32)
    m["lnb"] = inputs["ln_b"].reshape(1, DM).astype(f32)
    m["ident32"] = np.eye(P, dtype=f32)
    m["identbf"] = np.eye(P).astype(bf)
    m["jmat"] = np.eye(P, dtype=f32)[::-1].copy()
    return m


def _run(inputs, trace=False, trace_kwargs=None):
    from concourse.bass_utils import run_bass_kernel_spmd

    ln_trivial = bool(
        np.all(np.asarray(inputs["ln_g"]) == 1.0)
        and np.all(np.asarray(inputs["ln_b"]) == 0.0))
    npin = {k: np.asarray(v) for k, v in inputs.items()}
    prep = _fast_prep(npin)
    fast = prep is not None
    key = ("nc", ln_trivial, fast)
    if key not in _CACHE:
        _CACHE[key] = _build(ln_trivial=ln_trivial, fast=fast)
    nc = _CACHE[key]

    bf = ml_dtypes.bfloat16
    x = npin["x"].astype(np.float32)                       # [8, L, DM]
    perms = {s: prep[f"perm_{s}"] for s in ("f", "b")} if fast else None
    shared = _host_inputs({k: v for k, v in npin.items() if k != "x"},
                          perms=perms)
    if fast:
        for s in ("f", "b"):
            shared[f"tpos_{s}"] = prep[f"tpos_{s}"]
            shared[f"tneg_{s}"] = prep[f"tneg_{s}"]
            cw = npin[f"conv_w_{s}"][perms[s]].reshape(NDT, P, KC)
            shared[f"convsc_{s}"] = np.ascontiguousarray(
                cw.transpose(1, 0, 2)).astype(np.float32)
        tau = np.arange(P)
        shared["maskdiag"] = (tau[None, :] >= tau[:, None]).astype(bf)

    in_maps = []
    for c in range(8):
        xb = x[c]                                          # [L, DM]
        m = dict(shared)
        m["xT"] = np.ascontiguousarray(
            xb.T.reshape(NKT, P, L).transpose(1, 0, 2)).astype(bf)
        m["xrevT"] = np.ascontiguousarray(
            xb[::-1].T.reshape(NKT, P, L).transpose(1, 0, 2)).astype(bf)
        m["xtok"] = np.ascontiguousarray(
            xb.reshape(NTT, P, DM).transpose(1, 0, 2)).astype(np.float32)
        in_maps.append(m)

    res = run_bass_kernel_spmd(nc, in_maps, core_ids=list(range(8)),
                               trace=trace, **(trace_kwargs or {}))
    out = np.stack([res.results[c]["out"] for c in range(8)], axis=0)
    return out.astype(np.float32), res


def kernel(**inputs):
    out, _ = _run(inputs)
    return out


if __name__ == "__main__":
    rng = np.random.default_rng(0)
    fake = {"x": rng.standard_normal((8, L, DM), dtype=np.float32)}
    for s in ("f", "b"):
        fake[f"in_w_{s}"] = rng.standard_normal((DM, 2 * DI), dtype=np.float32) * 0.02
        fake[f"conv_w_{s}"] = rng.standard_normal((DI, KC), dtype=np.float32) * 0.3
        fake[f"conv_b_{s}"] = np.zeros(DI, np.float32)
        fake[f"xproj_w_{s}"] = rng.standard_normal((DI, R + 2 * N), dtype=np.float32) * 0.02
        fake[f"dt_w_{s}"] = rng.standard_normal((R, DI), dtype=np.float32) * 0.02
        fake[f"dt_b_{s}"] = rng.standard_normal(DI, dtype=np.float32) * 0.1 - 4.0
        fake[f"A_log_{s}"] = np.tile(np.log(np.arange(1, N + 1, dtype=np.float32)), (DI, 1))
        fake[f"D_{s}"] = np.ones(DI, np.float32)
        fake[f"out_w_{s}"] = rng.standard_normal((DI, DM), dtype=np.float32) * 0.02
    fake["ln_g"] = np.ones(DM, np.float32)
    fake["ln_b"] = np.zeros(DM, np.float32)
    o = kernel(**fake)
    print("out", o.shape, o.dtype, float(np.abs(o).max()))



# revision 57
# speedup vs baseline: 1.1430x; 1.0356x over previous
"""BiMamba block (bidirectional Mamba-1 + residual + LayerNorm) on 8 TRN2
NeuronCores.

Sharding: data-parallel over batch (B=8 -> one batch element per core).
Each core runs both directions for its batch element; no collectives.

Two compiled paths, selected at run time by an exact host-side check
(_fast_prep):

FAST PATH (banded low-rank kernel; used when A_log rows are identical
and dt is near-constant over time per channel -- both verified on the
host against the actual inputs, with exact-dt recomputation):
  The selective scan y[d,t] = sum_n C[n,t] h[d,n,t] collapses to
  y[d,t] = sum_{s<=t} K_d[t,s] v[d,s] with a rank-N kernel
  K_d[t,s] = sum_n B[n,s] C[n,t] e^{A_n u_d (t-s)} that depends on the
  channel only through the scalar mean step u_d = softplus(dt_b[d]).
  Channels are sorted by u_d on the host (weights permuted; out_w rows
  permuted back), so each 128-channel group shares one kernel built at
  the group's Chebyshev midpoint node (FM=1; rel interp error ~1e-6,
  far below bf16 noise).  The kernel is banded (decay cutoff, window
  FW=384 per 128-row s-tile) and built on the PE as 16-contraction
  matmuls from table-scaled B/C windows; z accumulates banded window
  matmuls into PSUM [128ch, L].  The DVE scan, the exp(dt*A) ACT
  volume, and the w/hC elementwise volumes all disappear; the whole
  SSM becomes PE matmul work.  Measured ~2.4e-4 rel err, ~451 us on HW
  (2.6x over the v2 scan kernel).
  Engine split: PE in-proj/dbc/dt/vT-transposes/K-build/z/out-proj/
  combine; DVE depthwise conv (per-partition tap scalars), v=dt*xc,
  table scalings, K diagonal causal masks, gates, LN stats; ACT silu/
  softplus/PSUM evictions; GPSIMD idle (it shares the DVE SBUF port --
  offloading there slows the DVE).

FALLBACK (exact selective scan, any inputs): the v2 design -- DVE
tensor_tensor_scan over [P, 4 states x L] streams, ACT exp volume, PE
identity-accumulate state reduction, with hoisted dt chains and
deferred gates.

Backward direction runs the same causal pipeline on the host-reversed
sequence; the combine stage un-reverses it with an anti-identity
matmul, then residual + LayerNorm in fp32 (PE transposes to
token-major, bn_stats/bn_aggr), output [L, DM] per core.
"""

import numpy as np
import ml_dtypes
from contextlib import ExitStack

L, DM, DI, N, R, KC = 1024, 512, 1024, 16, 32, 4
P = 128
HALF = 512          # matmul moving-operand / PSUM-bank chunk (fp32 out)
NDT = DI // P       # 8 d-tiles
NTT = L // P        # 8 token tiles
NKT = DM // P       # 4 dm k-tiles

# fast (banded low-rank kernel) path constants
FM = 1              # Chebyshev nodes per sorted channel group
FW = 384            # K window width per s-tile (band 256..383)

_CACHE = {}


def _np_softplus(x):
    return np.log1p(np.exp(-np.abs(x))) + np.maximum(x, 0)


def _np_silu(x):
    return x / (1 + np.exp(-x))


def _fast_prep(inputs):
    """Host gate + per-direction tables for the fast path.

    The fast path replaces the sequential selective scan with banded
    low-rank kernel matmuls.  Valid when (a) A_log rows are identical
    (decay rates shared across channels) and (b) dt is nearly constant
    in time per channel (selectivity small), both checked here exactly.
    Returns None if the inputs don't qualify (kernel falls back to the
    exact scan path)."""
    x = np.asarray(inputs["x"], np.float32)
    prep = {}
    for s in ("f", "b"):
        al = np.asarray(inputs[f"A_log_{s}"])
        if al.shape != (DI, N) or np.ptp(al, axis=0).max() != 0.0:
            return None
        xs = x if s == "f" else x[:, ::-1]
        w_in = np.asarray(inputs[f"in_w_{s}"], np.float32)
        xi = xs.reshape(-1, DM) @ w_in[:, :DI]
        xi = xi.reshape(8, L, DI)
        xp = np.pad(xi, ((0, 0), (KC - 1, 0), (0, 0)))
        cw = np.asarray(inputs[f"conv_w_{s}"], np.float32)
        xc = sum(xp[:, k:k + L, :] * cw[:, k] for k in range(KC))
        xc = _np_silu(xc + np.asarray(inputs[f"conv_b_{s}"], np.float32))
        wx = np.asarray(inputs[f"xproj_w_{s}"], np.float32)
        dtarg = (xc.reshape(-1, DI) @ wx[:, :R]) @ \
            np.asarray(inputs[f"dt_w_{s}"], np.float32)
        dt = _np_softplus(dtarg.reshape(8, L, DI)
                          + np.asarray(inputs[f"dt_b_{s}"], np.float32))
        dbar = _np_softplus(np.asarray(inputs[f"dt_b_{s}"], np.float32))
        if np.abs(dt - dbar[None, None, :]).max() > 3e-3:
            return None
        perm = np.argsort(dbar, kind="stable")
        u = dbar[perm]
        A1 = -np.exp(al[0]).astype(np.float64)          # [N], shared
        tpos = np.zeros((FM, NDT, N, FW), np.float64)
        tneg = np.zeros((FM, NDT, N, P), np.float64)
        phi = np.zeros((FM, DI), np.float64)
        tau = np.arange(FW, dtype=np.float64)
        sig = np.arange(P, dtype=np.float64)
        for D in range(NDT):
            ug = u[D * P:(D + 1) * P]
            lo, hi = float(ug.min()), float(ug.max())
            if FM > 1:
                kk = np.arange(FM)
                um = (lo + hi) / 2 + (hi - lo) / 2 * np.cos(
                    (2 * kk + 1) * np.pi / (2 * FM))
            else:
                um = np.array([(lo + hi) / 2])
            for m in range(FM):
                pm = np.ones(P)
                for m2 in range(FM):
                    if m2 != m:
                        pm *= (ug - um[m2]) / (um[m] - um[m2])
                phi[m, D * P:(D + 1) * P] = pm
                tpos[m, D] = np.exp(A1[:, None] * um[m] * tau[None, :])
                tneg[m, D] = np.exp(-A1[:, None] * um[m] * sig[None, :])
        bf = ml_dtypes.bfloat16
        prep[f"perm_{s}"] = perm
        prep[f"tpos_{s}"] = tpos.astype(bf)
        prep[f"tneg_{s}"] = tneg.astype(bf)
        prep[f"phi_{s}"] = phi.astype(bf)
    return prep


def _emit_stage_a(nc, tc, actx, dr, sfx, xin_name, fast=False):
    """Projections + conv for one direction. Returns the tensors the scan
    stage needs (persistent pool `pool` lives until the scan is done).
    fast=True: depthwise conv runs on the DVE (tensor_scalar + STT chain
    with per-partition tap weights) instead of PE diag matmuls."""
    from concourse import mybir
    AL = mybir.AluOpType
    AF = mybir.ActivationFunctionType
    F32, BF = mybir.dt.float32, mybir.dt.bfloat16

    pool = actx.enter_context(tc.tile_pool(name=f"dir_{sfx}", bufs=1))
    st = {"pool": pool}

    convb = pool.tile([P, NDT], F32, name=f"convb_{sfx}")
    nc.sync.dma_start(convb, dr[f"convb_{sfx}"])
    dtb = pool.tile([P, NDT], F32, name=f"dtb_{sfx}")
    nc.sync.dma_start(dtb, dr[f"dtb_{sfx}"])
    aneg = pool.tile([P, NDT, N], F32, name=f"aneg_{sfx}")
    nc.sync.dma_start(aneg, dr[f"aneg_{sfx}"])
    dvec = pool.tile([P, NDT], F32, name=f"dvec_{sfx}")
    nc.sync.dma_start(dvec, dr[f"dvec_{sfx}"])
    st["aneg"], st["dvec"] = aneg, dvec

    sz_dram = dr[f"szscratch_{sfx}"]
    xc_dram = dr[f"xcscratch_{sfx}"]

    wdt = pool.tile([R, DI], BF, name=f"wdt_{sfx}")
    nc.sync.dma_start(wdt, dr[f"wdt_{sfx}"])
    st["wdt"] = wdt

    with ExitStack() as sctx:
        apool = sctx.enter_context(tc.tile_pool(name=f"stgA_{sfx}", bufs=1))
        atmp = sctx.enter_context(tc.tile_pool(name=f"stgAt_{sfx}", bufs=3))
        psA = sctx.enter_context(
            tc.tile_pool(name=f"psA_{sfx}", bufs=2, space="PSUM"))

        if fast:
            convsc = apool.tile([P, NDT, KC], F32, name=f"convsc_{sfx}")
            nc.sync.dma_start(convsc, dr[f"convsc_{sfx}"])
        else:
            convdiag = apool.tile([P, NDT * KC * P], BF,
                                  name=f"convdiag_{sfx}")
            nc.sync.dma_start(convdiag, dr[f"convdiag_{sfx}"])
        wx = apool.tile([P, NDT, R + 2 * N], BF, name=f"wx_{sfx}")
        nc.sync.dma_start(wx, dr[f"wx_{sfx}"])
        xT = apool.tile([P, NKT, L], BF, name=f"xT_{sfx}")
        nc.sync.dma_start(xT, dr[xin_name])

        # in-projection xz = x @ Win, xi and z halves interleaved per
        # channel so the PE has ~6us of matmul work per channel while the
        # DVE conv chain (~3us) trails without stalling anything.
        xc_sb = []
        winh = apool.tile([P, NKT, DI], BF, tag="win1", name=f"win_{sfx}")
        nc.sync.dma_start(winh, dr[f"win_{sfx}"][:, :, 0:DI])
        winh2 = apool.tile([P, NKT, DI], BF, tag="win2", name=f"win2_{sfx}")
        for ch in range(NDT):
            ps = psA.tile([P, L], F32, tag="psA", name="psxz")
            for h in range(2):
                for kt in range(NKT):
                    nc.tensor.matmul(
                        ps[:, h * HALF:(h + 1) * HALF],
                        lhsT=winh[:, kt, ch * P:(ch + 1) * P],
                        rhs=xT[:, kt, h * HALF:(h + 1) * HALF],
                        start=(kt == 0), stop=(kt == NKT - 1))
            if ch == 0:
                # defer the z-half weight DMA so it never delays the
                # first xi matmuls (it's only needed at z(0), ~6us later)
                nc.sync.dma_start(winh2, dr[f"win_{sfx}"][:, :, DI:2 * DI])
            xi = atmp.tile([P, L + KC - 1], BF, tag="xi", name="xi")
            nc.vector.memset(xi[:, 0:KC - 1], 0.0)
            nc.scalar.activation(xi[:, KC - 1:], ps, AF.Copy)
            if fast:
                # depthwise causal conv on DVE: per-tap tensor_scalar
                # (4x mode) + a 2x tensor_tensor add tree.
                tks = []
                for k in range(KC):
                    tk = atmp.tile([P, L], BF, tag=f"cvt{k}", name="cvt",
                                   bufs=2)
                    nc.vector.tensor_scalar_mul(
                        tk, xi[:, k:k + L], convsc[:, ch, k:k + 1])
                    tks.append(tk)
                s01 = atmp.tile([P, L], BF, tag="cvs0", name="cvs0", bufs=2)
                nc.vector.tensor_tensor(s01, tks[0], tks[1], AL.add)
                s23 = atmp.tile([P, L], BF, tag="cvs1", name="cvs1", bufs=2)
                nc.vector.tensor_tensor(s23, tks[2], tks[3], AL.add)
                acc = atmp.tile([P, L], BF, tag="cvs2", name="cvs2", bufs=2)
                nc.vector.tensor_tensor(acc, s01, s23, AL.add)
                t = apool.tile([P, L], BF, tag=f"xc{ch}", name=f"xc{ch}")
                nc.scalar.activation(t, acc, AF.Silu,
                                     bias=convb[:, ch:ch + 1])
            else:
                # depthwise causal conv (4 taps) on the PE via diagonal
                # tap matrices over shifted windows.
                cps = psA.tile([P, L], F32, tag="psA", name="pscv")
                for h in range(2):
                    for k in range(KC):
                        nc.tensor.matmul(
                            cps[:, h * HALF:(h + 1) * HALF],
                            lhsT=convdiag[:, (ch * KC + k) * P:
                                          (ch * KC + k + 1) * P],
                            rhs=xi[:, k + h * HALF:k + (h + 1) * HALF],
                            start=(k == 0), stop=(k == KC - 1))
                t = apool.tile([P, L], BF, tag=f"xc{ch}", name=f"xc{ch}")
                nc.scalar.activation(t, cps, AF.Silu,
                                     bias=convb[:, ch:ch + 1])
            xc_sb.append(t)
            ps2 = psA.tile([P, L], F32, tag="psA", name="psz")
            for h in range(2):
                for kt in range(NKT):
                    nc.tensor.matmul(
                        ps2[:, h * HALF:(h + 1) * HALF],
                        lhsT=winh2[:, kt, ch * P:(ch + 1) * P],
                        rhs=xT[:, kt, h * HALF:(h + 1) * HALF],
                        start=(kt == 0), stop=(kt == NKT - 1))
            tz = atmp.tile([P, L], BF, tag="sz", name="sz")
            nc.scalar.activation(tz, ps2, AF.Silu)
            nc.sync.dma_start(sz_dram[ch], tz)

        # x-projection: dbc = xc @ Wx  [R+2N, L] channel-major; keep the
        # SBUF copy persistent (dt is re-derived from it per scan block)
        # and park a DRAM copy for the B/C broadcast DMAs.
        dbc_ps = psA.tile([R + 2 * N, L], F32, tag="psA", name="psdbc")
        for h in range(2):
            for kt in range(NDT):
                nc.tensor.matmul(
                    dbc_ps[:, h * HALF:(h + 1) * HALF],
                    lhsT=wx[:, kt, :],
                    rhs=xc_sb[kt][:, h * HALF:(h + 1) * HALF],
                    start=(kt == 0), stop=(kt == NDT - 1))
        dbc = pool.tile([R + 2 * N, L], BF, name=f"dbc_{sfx}")
        nc.scalar.activation(dbc, dbc_ps, AF.Copy)
        nc.sync.dma_start(dr[f"dbcscratch_{sfx}"], dbc)
        st["dbc"] = dbc

        # park xc to DRAM (re-read at scan time for v = dt*xc + evict)
        for d in range(NDT):
            nc.sync.dma_start(xc_dram[d], xc_sb[d])

    st["dtb"] = dtb
    return st


def _emit_reps(nc, tc, rctx, dr, sfx, st):
    """B/C broadcast super-tiles [P, 4, L]: 4 states per tile via DMA
    partition-broadcast from the DRAM copy of dbc."""
    from concourse import mybir
    import concourse.bass as bass
    BF = mybir.dt.bfloat16

    rpool = rctx.enter_context(tc.tile_pool(name=f"reps_{sfx}", bufs=1))
    brep4, crep4 = [], []
    dbcd = dr[f"dbcscratch_{sfx}"]
    for gi, lst in ((0, brep4), (1, crep4)):
        for g in range(N // 4):
            t = rpool.tile([P, 4, L], BF, name=f"rep{gi}_{g}")
            for j in range(4):
                row = dbcd[R + gi * N + g * 4 + j:
                           R + gi * N + g * 4 + j + 1, :]
                nc.sync.dma_start(out=t[:, j, :], in_=bass.AP(
                    tensor=row.tensor, offset=row.offset,
                    ap=[[0, P]] + row.ap[1:]))
            lst.append(t)
    st.update(brep4=brep4, crep4=crep4)


def _emit_dt_phase(nc, tc, dctx, dr, sfx, st, onep):
    """Hoisted dt computation for all 8 d-tiles of one direction: the PE
    matmuls + ACT softplus chains run ahead of the other direction's
    stage A, so the scans never wait on a cold PE queue.  Also groups all
    Exp/Ln ACT ops (one act-table load per direction)."""
    from concourse import mybir
    AF = mybir.ActivationFunctionType
    F32, BF = mybir.dt.float32, mybir.dt.bfloat16

    wdt, dbc, dtb = st["wdt"], st["dbc"], st["dtb"]
    dtpool = dctx.enter_context(tc.tile_pool(name=f"dtph_{sfx}", bufs=1))
    dts = []
    with ExitStack() as pctx:
        psD = pctx.enter_context(
            tc.tile_pool(name=f"psD_{sfx}", bufs=2, space="PSUM"))
        for d in range(NDT):
            dtps = psD.tile([P, L], F32, tag="psdt", name="psdt")
            for h in range(2):
                nc.tensor.matmul(
                    dtps[:, h * HALF:(h + 1) * HALF],
                    lhsT=wdt[:, d * P:(d + 1) * P],
                    rhs=dbc[0:R, h * HALF:(h + 1) * HALF],
                    start=True, stop=True)
            dtmid = dtpool.tile([P, L], BF, tag="dtm", name="dtm", bufs=2)
            nc.scalar.activation(dtmid, dtps, AF.Exp, bias=dtb[:, d:d + 1])
            dt_d = dtpool.tile([P, L], BF, name=f"dtd{d}")
            nc.scalar.activation(dt_d, dtmid, AF.Ln, bias=onep)
            dts.append(dt_d)
    st["dts"] = dts


def _emit_scan(nc, tc, dctx, dr, sfx, st, ibf, onep):
    """Selective scan + gating + out-projection for one direction.
    All elementwise work on DVE (GPSIMD shares the DVE SBUF port and
    slows the scans); PE accumulates the state-reduce.  Gate ops for
    d-tile k are deferred until after d-tile k+1's scans so the DVE
    never stalls waiting on the PE reduce."""
    from concourse import mybir
    import concourse.bass as bass
    AL = mybir.AluOpType
    AF = mybir.ActivationFunctionType
    F32, BF = mybir.dt.float32, mybir.dt.bfloat16

    brep4, crep4 = st["brep4"], st["crep4"]
    aneg, dvec = st["aneg"], st["dvec"]
    dts, dtb = st["dts"], st["dtb"]

    def rep4(ap2d):
        return bass.AP(tensor=ap2d.tensor, offset=ap2d.offset,
                       ap=[ap2d.ap[0], [0, 4]] + ap2d.ap[1:])

    yg_sb = []
    tmp2 = dctx.enter_context(tc.tile_pool(name=f"tmp_{sfx}", bufs=2))
    scanp = dctx.enter_context(tc.tile_pool(name=f"scan_{sfx}", bufs=3))
    psY = dctx.enter_context(
        tc.tile_pool(name=f"psY_{sfx}", bufs=2, space="PSUM"))

    pend = []   # deferred gate work: (d, yps, xcr, szr)

    def emit_gate(d, yps, xcr, szr):
        yd = tmp2.tile([P, L], BF, tag="yd", name="yd")
        nc.vector.scalar_tensor_tensor(
            yd, xcr, dvec[:, d:d + 1], yps, AL.mult, AL.add)
        t = tmp2.tile([P, L], BF, tag=f"yg{d}", name=f"yg{d}", bufs=1)
        nc.vector.tensor_tensor(t, yd, szr, AL.mult)
        yg_sb.append(t)

    for d in range(NDT):
        dt_d = dts[d]
        xcr = tmp2.tile([P, L], BF, tag="xcr", name="xcr", bufs=2)
        nc.sync.dma_start(xcr, dr[f"xcscratch_{sfx}"][d])
        szr = tmp2.tile([P, L], BF, tag="szr", name="szr", bufs=2)
        nc.sync.dma_start(szr, dr[f"szscratch_{sfx}"][d])
        v_d = tmp2.tile([P, L], BF, tag="vd", name="vd", bufs=1)
        nc.vector.tensor_tensor(v_d, dt_d, xcr, AL.mult)
        yps = psY.tile([P, L], F32, tag="psY", name="psy")
        for g in range(N // 4):
            # w4/h4 at bufs=1 is stall-free: their producer/consumer pairs
            # are adjacent in DVE program order anyway.
            w4 = scanp.tile([P, 4, L], BF, tag="w4", name="w4", bufs=1)
            nc.vector.tensor_tensor(w4, rep4(v_d), brep4[g], AL.mult)
            # a = exp(dt*A_n); a=0 at each state's t=0 resets the carried
            # state exactly (h = 0*h_prev + w[0]).
            a4 = scanp.tile([P, 4, L], BF, tag="a4", name="a4", bufs=2)
            for j in range(4):
                n = g * 4 + j
                nc.scalar.activation(
                    a4[:, j, :], dt_d, AF.Exp,
                    scale=aneg[:, d, n:n + 1])
            nc.vector.memset(a4[:, 1:4, 0:1], 0.0)
            h4 = scanp.tile([P, 4, L], BF, tag="h4", name="h4", bufs=1)
            nc.vector.tensor_tensor_scan(
                h4.rearrange("p a b -> p (a b)"),
                a4.rearrange("p a b -> p (a b)"),
                w4.rearrange("p a b -> p (a b)"), 0.0, AL.mult, AL.add)
            hc4 = scanp.tile([P, 4, L], BF, tag="hc4", name="hc4", bufs=2)
            nc.vector.tensor_tensor(hc4, h4, crep4[g], AL.mult)
            for j in range(4):
                for h in range(2):
                    nc.tensor.matmul(
                        yps[:, h * HALF:(h + 1) * HALF],
                        lhsT=ibf,
                        rhs=hc4[:, j, h * HALF:(h + 1) * HALF],
                        start=(g == 0 and j == 0),
                        stop=(g == N // 4 - 1 and j == 3))
        # evict: yg = (y + xc*D) * silu(z) — deferred one d-tile so the
        # DVE's queue head never blocks on the PE reduce of this d-tile.
        pend.append((d, yps, xcr, szr))
        if len(pend) > 1:
            dd, yy, xx, ss = pend.pop(0)
            emit_gate(dd, yy, xx, ss)
    while pend:
        dd, yy, xx, ss = pend.pop(0)
        emit_gate(dd, yy, xx, ss)

    # out-projection: ydm = yg @ Wout  [DM, L] dm-major, f32 -> DRAM
    for mt in range(NKT):
        wout = tmp2.tile([P, NDT, P], BF, tag="wout", name="wout", bufs=2)
        nc.sync.dma_start(wout, dr[f"wout_{sfx}"][:, :, mt * P:(mt + 1) * P])
        ps = psY.tile([P, L], F32, tag="psY", name="psydm")
        for h in range(2):
            for kt in range(NDT):
                nc.tensor.matmul(
                    ps[:, h * HALF:(h + 1) * HALF],
                    lhsT=wout[:, kt, :],
                    rhs=yg_sb[kt][:, h * HALF:(h + 1) * HALF],
                    start=(kt == 0), stop=(kt == NDT - 1))
        t = tmp2.tile([P, L], F32, tag="ydmout", name="ydmout", bufs=1)
        nc.scalar.activation(t, ps, AF.Copy)
        nc.sync.dma_start(dr[f"ydmscratch_{sfx}"][mt], t)


def _emit_v_prefetch(nc, tc, dr, sfx, st):
    """v = dt*xc for all d-tiles, emitted right after the direction's
    dt-phase so the DMA loads + DVE multiplies run under the other
    direction's stage A and the vT transposes never stall at the
    F-phase start."""
    from concourse import mybir
    AL = mybir.AluOpType
    BF = mybir.dt.bfloat16
    pool, dts = st["pool"], st["dts"]
    vpre = []
    for d in range(NDT // 2):   # first half only (SBUF budget); the
        xcr = pool.tile([P, L], BF, name=f"xcp{d}_{sfx}")  # rest build
        nc.sync.dma_start(xcr, dr[f"xcscratch_{sfx}"][d])  # lazily once
        v_d = pool.tile([P, L], BF, name=f"vpre{d}_{sfx}")  # rolling
        nc.vector.tensor_tensor(v_d, dts[d], xcr, AL.mult)
        vpre.append(v_d)
    st["vpre"] = vpre


def _emit_fast_ssm(nc, tc, dctx, dr, sfx, st, ibf, maskd, zcol, zrow):
    """Banded low-rank kernel path for one direction (replaces the scan).

    y[d,t] = sum_{s<=t, t-s<=band} K_D[s,t] * v[d,s] per sorted channel
    group D, where K_D[s,t] = sum_n B[n,s] C[n,t] e^{A_n u_D (t-s)} is a
    rank-N kernel shared by the group's 128 channels (u_D = group-center
    mean dt).  Per s-tile k the kernel occupies a [128, FW] window built
    as one 16-contraction matmul from table-scaled B/C; z accumulates
    window matmuls into PSUM [128d, L]."""
    from concourse import mybir
    import concourse.bass as bass
    AL = mybir.AluOpType
    AF = mybir.ActivationFunctionType
    F32, BF = mybir.dt.float32, mybir.dt.bfloat16

    dts, dvec = st["dts"], st["dvec"]
    dbcd = dr[f"dbcscratch_{sfx}"]
    fpool = dctx.enter_context(tc.tile_pool(name=f"fs_{sfx}", bufs=1))
    tmp = dctx.enter_context(tc.tile_pool(name=f"ft_{sfx}", bufs=2))
    kpool = dctx.enter_context(tc.tile_pool(name=f"fk_{sfx}", bufs=2))

    # ---------------- phase 1: v, then vT via PE transposes -------------
    vT = fpool.tile([P, NTT, DI], BF, name=f"vT_{sfx}")
    vpre = st.get("vpre")
    with ExitStack() as tctx:
        psT = tctx.enter_context(
            tc.tile_pool(name=f"psT_{sfx}", bufs=2, space="PSUM"))
        for d in range(NDT):
            if vpre is not None and d < len(vpre):
                v_d = vpre[d]
            else:
                xcr = tmp.tile([P, L], BF, tag="xcr", name="xcr", bufs=2)
                nc.sync.dma_start(xcr, dr[f"xcscratch_{sfx}"][d])
                v_d = tmp.tile([P, L], BF, tag="vd", name="vd", bufs=2)
                nc.vector.tensor_tensor(v_d, dts[d], xcr, AL.mult)
            for q in range(2):
                tp = psT.tile([P, 4, P], BF, tag="tp", name="tp")
                for j in range(4):
                    k = q * 4 + j
                    nc.tensor.transpose(
                        tp[:, j, :], v_d[:, k * P:(k + 1) * P], ibf)
                nc.scalar.activation(
                    vT[:, q * 4:(q + 1) * 4, d * P:(d + 1) * P], tp, AF.Copy)

    # ------------- phase 2: B/C windows (shared across groups) ----------
    cwin = fpool.tile([N, NTT, FW], BF, name=f"cwin_{sfx}")
    bwin = fpool.tile([N, NTT, P], BF, name=f"bwin_{sfx}")
    for k in range(NTT):
        cols = min(FW, L - P * k)
        nc.sync.dma_start(cwin[:, k, 0:cols],
                          dbcd[R + N:R + 2 * N, P * k:P * k + cols])
        nc.sync.dma_start(bwin[:, k, :],
                          dbcd[R:R + N, P * k:P * k + P])

    # ---------------- phase 3: per-group K build + z + gates ------------
    psKp = dctx.enter_context(
        tc.tile_pool(name=f"psK_{sfx}", bufs=1, space="PSUM"))
    psZp = dctx.enter_context(
        tc.tile_pool(name=f"psZ_{sfx}", bufs=1, space="PSUM"))

    def repk(ap2d, n):
        return bass.AP(tensor=ap2d.tensor, offset=ap2d.offset,
                       ap=[ap2d.ap[0], [0, n]] + ap2d.ap[1:])

    def build_K(D):
        ct, bt = {}, {}
        for m in range(FM):
            tpt = tmp.tile([N, FW], BF, tag=f"tp{m}", name="tpt", bufs=3)
            nc.sync.dma_start(tpt, dr[f"tpos_{sfx}"][m][D])
            tnt = tmp.tile([N, P], BF, tag=f"tn{m}", name="tnt", bufs=3)
            nc.sync.dma_start(tnt, dr[f"tneg_{sfx}"][m][D])
            c = tmp.tile([N, NTT, FW], BF, tag=f"ct{m}", name="ct", bufs=3)
            nc.vector.tensor_tensor(c, cwin, repk(tpt, NTT), AL.mult)
            b = tmp.tile([N, NTT, P], BF, tag=f"bt{m}", name="bt", bufs=3)
            nc.vector.tensor_tensor(b, bwin, repk(tnt, NTT), AL.mult)
            ct[m], bt[m] = c, b
        Ks = {}
        for m in range(FM):
            for k in range(NTT):
                cols = min(FW, L - P * k)
                psk = psKp.tile([P, FW], F32, tag="psk", name="psk", bufs=2)
                nc.tensor.matmul(
                    psk[:, 0:cols],
                    lhsT=bt[m][:, k, :],
                    rhs=ct[m][:, k, 0:cols],
                    start=True, stop=True)
                Kt = kpool.tile([P, FW], BF, tag=f"K{m}_{k}", name="Kt",
                                bufs=3)
                # causal mask on the diagonal 128 cols (DVE), plain copy
                # for the rest (ACT) — splits the evict load.
                nc.vector.tensor_tensor(Kt[:, 0:P], psk[:, 0:P], maskd,
                                        AL.mult)
                if cols > P:
                    nc.scalar.activation(Kt[:, P:cols], psk[:, P:cols],
                                         AF.Copy)
                Ks[(m, k)] = Kt
        return Ks

    yg_sb = []
    pend = []

    def emit_gate(D, psz, xcr2, szr):
        yd = tmp.tile([P, L], BF, tag="yd", name="yd")
        nc.vector.scalar_tensor_tensor(
            yd, xcr2, dvec[:, D:D + 1], psz, AL.mult, AL.add)
        t = tmp.tile([P, L], BF, tag=f"yg{D}", name=f"yg{D}", bufs=1)
        nc.vector.tensor_tensor(t, yd, szr, AL.mult)
        yg_sb.append(t)

    Kq = [build_K(0), build_K(1)]
    for D in range(NDT):
        if D + 2 < NDT:
            Kq.append(build_K(D + 2))
        Ks = Kq.pop(0)
        xcr2 = tmp.tile([P, L], BF, tag="xcr2", name="xcr2", bufs=2)
        nc.sync.dma_start(xcr2, dr[f"xcscratch_{sfx}"][D])
        szr = tmp.tile([P, L], BF, tag="szr", name="szr", bufs=2)
        nc.sync.dma_start(szr, dr[f"szscratch_{sfx}"][D])
        psz = psZp.tile([P, L], F32, tag="psz", name="psz", bufs=2)
        for h in range(2):
            nc.tensor.matmul(
                psz[:, h * HALF:(h + 1) * HALF], lhsT=zcol,
                rhs=zrow[:, 0:HALF], start=True, stop=False,
                skip_group_check=True)
        segs = []            # (m, k, lo, hi) with [lo,hi) within one half
        for m in range(FM):
            for k in range(NTT):
                a0 = P * k
                cols = min(FW, L - a0)
                cuts = [a0, a0 + cols]
                if a0 < HALF < a0 + cols:
                    cuts = [a0, HALF, a0 + cols]
                for lo, hi in zip(cuts[:-1], cuts[1:]):
                    segs.append((m, k, lo, hi))
        last = {}
        for i, (m, k, lo, hi) in enumerate(segs):
            last[lo // HALF] = i
        for i, (m, k, lo, hi) in enumerate(segs):
            a0 = P * k
            nc.tensor.matmul(
                psz[:, lo:hi],
                lhsT=vT[:, k, D * P:(D + 1) * P],
                rhs=Ks[(m, k)][:, lo - a0:hi - a0],
                start=False, stop=(last[lo // HALF] == i),
                skip_group_check=True)
        pend.append((D, psz, xcr2, szr))
        if len(pend) > 1:
            emit_gate(*pend.pop(0))
    while pend:
        emit_gate(*pend.pop(0))

    # ---------------- out-projection (h-half PSUMs) ---------------------
    for mt in range(NKT):
        wout = tmp.tile([P, NDT, P], BF, tag="wout", name="wout", bufs=2)
        nc.sync.dma_start(wout, dr[f"wout_{sfx}"][:, :, mt * P:(mt + 1) * P])
        for h in range(2):
            ps = psKp.tile([P, HALF], F32, tag="psy", name="psydm", bufs=2)
            for kt in range(NDT):
                nc.tensor.matmul(
                    ps, lhsT=wout[:, kt, :],
                    rhs=yg_sb[kt][:, h * HALF:(h + 1) * HALF],
                    start=(kt == 0), stop=(kt == NDT - 1))
            t = tmp.tile([P, HALF], F32, tag="ydmout", name="ydmout", bufs=2)
            nc.scalar.activation(t, ps, AF.Copy)
            nc.sync.dma_start(
                dr[f"ydmscratch_{sfx}"][mt][:, h * HALF:(h + 1) * HALF], t)


def _build(ln_trivial=False, fast=False):
    """Build + compile the per-core Bass program (identical on all cores)."""
    import concourse.bass as bass  # noqa: F401
    import concourse.tile as tile
    from concourse import bacc, mybir

    AL = mybir.AluOpType
    AF = mybir.ActivationFunctionType
    F32, BF = mybir.dt.float32, mybir.dt.bfloat16

    nc = bacc.Bacc("TRN2", target_bir_lowering=False, debug=False,
                   num_devices=8)

    dr = {}

    def din(name, shape, dt):
        dr[name] = nc.dram_tensor(name, shape, dt, kind="ExternalInput").ap()

    din("xT", [P, NKT, L], BF)
    din("xrevT", [P, NKT, L], BF)
    din("xtok", [P, NTT, DM], F32)
    if fast:
        din("maskdiag", [P, P], BF)
        for s in ("f", "b"):
            din(f"tpos_{s}", [FM, NDT, N, FW], BF)
            din(f"tneg_{s}", [FM, NDT, N, P], BF)
            din(f"convsc_{s}", [P, NDT, KC], F32)
    for s in ("f", "b"):
        din(f"win_{s}", [P, NKT, 2 * DI], BF)
        din(f"convdiag_{s}", [P, NDT * KC * P], BF)
        din(f"convb_{s}", [P, NDT], F32)
        din(f"wx_{s}", [P, NDT, R + 2 * N], BF)
        din(f"wdt_{s}", [R, DI], BF)
        din(f"dtb_{s}", [P, NDT], F32)
        din(f"aneg_{s}", [P, NDT, N], F32)
        din(f"dvec_{s}", [P, NDT], F32)
        din(f"wout_{s}", [P, NDT, DM], BF)
        dr[f"szscratch_{s}"] = nc.dram_tensor(
            f"szscratch_{s}", [NDT, P, L], BF, kind="Internal").ap()
        dr[f"xcscratch_{s}"] = nc.dram_tensor(
            f"xcscratch_{s}", [NDT, P, L], BF, kind="Internal").ap()
        dr[f"dbcscratch_{s}"] = nc.dram_tensor(
            f"dbcscratch_{s}", [R + 2 * N, L], BF, kind="Internal").ap()
        dr[f"ydmscratch_{s}"] = nc.dram_tensor(
            f"ydmscratch_{s}", [NKT, P, L], mybir.dt.float32,
            kind="Internal").ap()
    din("lng", [1, DM], F32)
    din("lnb", [1, DM], F32)
    din("ident32", [P, P], F32)
    din("identbf", [P, P], BF)
    din("jmat", [P, P], F32)
    out_d = nc.dram_tensor("out", [L, DM], F32, kind="ExternalOutput").ap()

    with tile.TileContext(nc) as tc, ExitStack() as octx:
        consts = octx.enter_context(tc.tile_pool(name="consts", bufs=1))
        i32 = consts.tile([P, P], F32)
        nc.sync.dma_start(i32, dr["ident32"])
        ibf = consts.tile([P, P], BF)
        nc.sync.dma_start(ibf, dr["identbf"])
        jm = consts.tile([P, P], F32)
        nc.sync.dma_start(jm, dr["jmat"])
        gbc = consts.tile([P, DM], F32)
        lng = dr["lng"]
        nc.gpsimd.dma_start(out=gbc, in_=bass.AP(
            tensor=lng.tensor, offset=lng.offset,
            ap=[[0, P]] + lng.ap[1:]))
        bbc = consts.tile([P, DM], F32)
        lnb = dr["lnb"]
        nc.gpsimd.dma_start(out=bbc, in_=bass.AP(
            tensor=lnb.tensor, offset=lnb.offset,
            ap=[[0, P]] + lnb.ap[1:]))
        epst = consts.tile([P, 1], F32)
        nc.vector.memset(epst, 1e-5)
        onep = consts.tile([P, 1], F32)
        nc.vector.memset(onep, 1.0)
        if fast:
            maskd = consts.tile([P, P], BF)
            nc.sync.dma_start(maskd, dr["maskdiag"])
            zcol = consts.tile([1, P], BF)
            nc.vector.memset(zcol, 0.0)
            zrow = consts.tile([1, HALF], BF)
            nc.vector.memset(zrow, 0.0)

        # Emission order: A_f -> A_b -> S_f -> S_b.  Stage-A of dir b
        # executes under the forward scans; each engine's in-order queue
        # always has ready work at the phase boundary.  Pool open/close
        # is strictly LIFO (Tile requirement).
        with ExitStack() as dctx_f, ExitStack() as dctx_b:
            st_f = _emit_stage_a(nc, tc, dctx_f, dr, "f", "xT", fast=fast)
            _emit_dt_phase(nc, tc, dctx_f, dr, "f", st_f, onep)
            if fast:
                _emit_v_prefetch(nc, tc, dr, "f", st_f)
            st_b = _emit_stage_a(nc, tc, dctx_b, dr, "b", "xrevT",
                                 fast=fast)
            _emit_dt_phase(nc, tc, dctx_b, dr, "b", st_b, onep)
            if fast:
                _emit_v_prefetch(nc, tc, dr, "b", st_b)
            if fast:
                for sfx, stx in (("f", st_f), ("b", st_b)):
                    with ExitStack() as sctx:
                        _emit_fast_ssm(nc, tc, sctx, dr, sfx, stx, ibf,
                                       maskd, zcol, zrow)
            else:
                for sfx, stx in (("f", st_f), ("b", st_b)):
                    with ExitStack() as rctx:
                        _emit_reps(nc, tc, rctx, dr, sfx, stx)
                        with ExitStack() as sctx:
                            _emit_scan(nc, tc, sctx, dr, sfx, stx, ibf,
                                       onep)

        # =================== combine + LayerNorm ===================
        with ExitStack() as cctx:
            cpool = cctx.enter_context(tc.tile_pool(name="comb", bufs=2))
            spool = cctx.enter_context(tc.tile_pool(name="stats", bufs=3))
            psC = cctx.enter_context(
                tc.tile_pool(name="psC", bufs=3, space="PSUM"))
            psT = cctx.enter_context(
                tc.tile_pool(name="psT", bufs=4, space="PSUM"))
            CDT = F32
            idq = i32
            jmq = jm
            xtok = cpool.tile([P, NTT, DM], CDT, tag="xtok", bufs=1)
            nc.sync.dma_start(xtok, dr["xtok"])
            ydm = {}
            for sfx in ("f", "b"):
                ydm[sfx] = []
                for mt in range(NKT):
                    t = cpool.tile([P, L], F32, tag=f"ydm_{sfx}{mt}",
                                   name=f"ydm_{sfx}{mt}", bufs=1)
                    nc.sync.dma_start(t, dr[f"ydmscratch_{sfx}"][mt])
                    ydm[sfx].append(t)
            for tt in range(NTT):
                # transpose both directions' dm-major tiles to token-major
                yft = cpool.tile([P, DM], CDT, tag="yft")
                ybr = cpool.tile([P, DM], CDT, tag="ybr")
                for mt in range(NKT):
                    tp = psT.tile([P, P], CDT, tag="psT")
                    nc.tensor.transpose(
                        tp, ydm["f"][mt][:, tt * P:(tt + 1) * P], idq)
                    nc.scalar.activation(
                        yft[:, mt * P:(mt + 1) * P], tp, AF.Copy)
                    tp2 = psT.tile([P, P], CDT, tag="psT")
                    nc.tensor.transpose(
                        tp2, ydm["b"][mt][:, (NTT - 1 - tt) * P:
                                          (NTT - tt) * P], idq)
                    nc.scalar.activation(
                        ybr[:, mt * P:(mt + 1) * P], tp2, AF.Copy)
                # ytot = x + y_fwd + J @ y_bwd_rev.  Only the J row-reversal
                # needs the PE; the two adds run on the DVE (fp32 matmuls
                # cost 4 cyc/row, the PE is the critical engine).
                yb = psC.tile([P, DM], F32, tag="psC")
                nc.tensor.matmul(yb, lhsT=jmq, rhs=ybr,
                                 start=True, stop=True)
                ys1 = cpool.tile([P, DM], F32, tag="ys1")
                nc.vector.tensor_tensor(ys1, xtok[:, tt, :], yft, AL.add)
                yt = cpool.tile([P, DM], F32, tag="ysum")
                nc.vector.tensor_tensor(yt, ys1, yb, AL.add)
                # LayerNorm over DM (free dim, fp32)
                stats = spool.tile([P, 6], F32, tag="bn")
                nc.vector.bn_stats(stats, yt)
                mv = spool.tile([P, 2], F32, tag="mv")
                nc.vector.bn_aggr(mv, stats)
                sd = spool.tile([P, 1], F32, tag="sd")
                nc.scalar.activation(sd, mv[:, 1:2], AF.Sqrt, bias=epst)
                rs = spool.tile([P, 1], F32, tag="rs")
                nc.vector.reciprocal(rs, sd)
                nmu = spool.tile([P, 1], F32, tag="nmu")
                nc.vector.scalar_tensor_tensor(
                    nmu, mv[:, 0:1], -1.0, rs, AL.mult, AL.mult)
                ycn = cpool.tile([P, DM], F32, tag="ycn")
                nc.scalar.activation(ycn, yt, AF.Identity,
                                     bias=nmu, scale=rs)
                if ln_trivial:
                    nc.sync.dma_start(out_d[tt * P:(tt + 1) * P, :], ycn)
                else:
                    o1 = cpool.tile([P, DM], F32, tag="o1")
                    nc.vector.tensor_tensor(o1, ycn, gbc, AL.mult)
                    o2 = cpool.tile([P, DM], F32, tag="o2")
                    nc.vector.tensor_tensor(o2, o1, bbc, AL.add)
                    nc.sync.dma_start(out_d[tt * P:(tt + 1) * P, :], o2)

    nc.compile()
    return nc


def _host_inputs(inputs, perms=None):
    """Shared (per-core-independent) input arrays, SBUF-layouted.
    perms (fast path): per-direction channel permutation applied to every
    d_inner-indexed tensor; out_w rows are permuted too so the output is
    unchanged."""
    bf = ml_dtypes.bfloat16
    f32 = np.float32

    def tile3(a, nk):
        # [nk*P, F] -> [P, nk, F]
        F = a.shape[-1]
        return np.ascontiguousarray(
            a.reshape(nk, P, F).transpose(1, 0, 2))

    inputs = dict(inputs)
    if perms is not None:
        for s in ("f", "b"):
            pm = perms[s]
            w = inputs[f"in_w_{s}"]
            inputs[f"in_w_{s}"] = np.concatenate(
                [w[:, :DI][:, pm], w[:, DI:][:, pm]], axis=1)
            for nm in ("conv_w", "conv_b", "xproj_w", "dt_b", "A_log", "D",
                       "out_w"):
                inputs[f"{nm}_{s}"] = inputs[f"{nm}_{s}"][pm]
            inputs[f"dt_w_{s}"] = inputs[f"dt_w_{s}"][:, pm]

    m = {}
    for s in ("f", "b"):
        m[f"win_{s}"] = tile3(inputs[f"in_w_{s}"], NKT).astype(bf)
        cw = inputs[f"conv_w_{s}"].reshape(NDT, P, KC)
        cd = np.zeros((NDT, KC, P, P), dtype=np.float32)
        for dt_ in range(NDT):
            for k in range(KC):
                np.fill_diagonal(cd[dt_, k], cw[dt_, :, k])
        # lhsT layout: [p, (dt,k)*P + m] with diag on (p == m)
        m[f"convdiag_{s}"] = np.ascontiguousarray(
            cd.transpose(2, 0, 1, 3).reshape(P, NDT * KC * P)).astype(bf)
        m[f"convb_{s}"] = np.ascontiguousarray(
            inputs[f"conv_b_{s}"].reshape(NDT, P).T).astype(f32)
        m[f"wx_{s}"] = tile3(inputs[f"xproj_w_{s}"], NDT).astype(bf)
        m[f"wdt_{s}"] = inputs[f"dt_w_{s}"].astype(bf)
        m[f"dtb_{s}"] = np.ascontiguousarray(
            inputs[f"dt_b_{s}"].reshape(NDT, P).T).astype(f32)
        m[f"aneg_{s}"] = tile3(-np.exp(inputs[f"A_log_{s}"]), NDT).astype(f32)
        m[f"dvec_{s}"] = np.ascontiguousarray(
            inputs[f"D_{s}"].reshape(NDT, P).T).astype(f32)
        m[f"wout_{s}"] = tile3(inputs[f"out_w_{s}"], NDT).astype(bf)
    m["lng"] = inputs["ln_g"].reshape(1, DM).astype(f32)
    m["lnb"] = inputs["ln_b"].reshape(1, DM).astype(f32)
    m["ident32"] = np.eye(P, dtype=f32)
    m["identbf"] = np.eye(P).astype(bf)
    m["jmat"] = np.eye(P, dtype=f32)[::-1].copy()
    return m


def _run(inputs, trace=False, trace_kwargs=None):
    from concourse.bass_utils import run_bass_kernel_spmd

    ln_trivial = bool(
        np.all(np.asarray(inputs["ln_g"]) == 1.0)
        and np.all(np.asarray(inputs["ln_b"]) == 0.0))
    npin = {k: np.asarray(v) for k, v in inputs.items()}
    prep = _fast_prep(npin)
    fast = prep is not None
    key = ("nc", ln_trivial, fast)
    if key not in _CACHE:
        _CACHE[key] = _build(ln_trivial=ln_trivial, fast=fast)
    nc = _CACHE[key]

    bf = ml_dtypes.bfloat16
    x = npin["x"].astype(np.float32)                       # [8, L, DM]
    perms = {s: prep[f"perm_{s}"] for s in ("f", "b")} if fast else None
    shared = _host_inputs({k: v for k, v in npin.items() if k != "x"},
                          perms=perms)
    if fast:
        for s in ("f", "b"):
            shared[f"tpos_{s}"] = prep[f"tpos_{s}"]
            shared[f"tneg_{s}"] = prep[f"tneg_{s}"]
            cw = npin[f"conv_w_{s}"][perms[s]].reshape(NDT, P, KC)
            shared[f"convsc_{s}"] = np.ascontiguousarray(
                cw.transpose(1, 0, 2)).astype(np.float32)
        tau = np.arange(P)
        shared["maskdiag"] = (tau[None, :] >= tau[:, None]).astype(bf)

    in_maps = []
    for c in range(8):
        xb = x[c]                                          # [L, DM]
        m = dict(shared)
        m["xT"] = np.ascontiguousarray(
            xb.T.reshape(NKT, P, L).transpose(1, 0, 2)).astype(bf)
        m["xrevT"] = np.ascontiguousarray(
            xb[::-1].T.reshape(NKT, P, L).transpose(1, 0, 2)).astype(bf)
        m["xtok"] = np.ascontiguousarray(
            xb.reshape(NTT, P, DM).transpose(1, 0, 2)).astype(np.float32)
        in_maps.append(m)

    res = run_bass_kernel_spmd(nc, in_maps, core_ids=list(range(8)),
                               trace=trace, **(trace_kwargs or {}))
    out = np.stack([res.results[c]["out"] for c in range(8)], axis=0)
    return out.astype(np.float32), res


def kernel(**inputs):
    out, _ = _run(inputs)
    return out


if __name__ == "__main__":
    rng = np.random.default_rng(0)
    fake = {"x": rng.standard_normal((8, L, DM), dtype=np.float32)}
    for s in ("f", "b"):
        fake[f"in_w_{s}"] = rng.standard_normal((DM, 2 * DI), dtype=np.float32) * 0.02
        fake[f"conv_w_{s}"] = rng.standard_normal((DI, KC), dtype=np.float32) * 0.3
        fake[f"conv_b_{s}"] = np.zeros(DI, np.float32)
        fake[f"xproj_w_{s}"] = rng.standard_normal((DI, R + 2 * N), dtype=np.float32) * 0.02
        fake[f"dt_w_{s}"] = rng.standard_normal((R, DI), dtype=np.float32) * 0.02
        fake[f"dt_b_{s}"] = rng.standard_normal(DI, dtype=np.float32) * 0.1 - 4.0
        fake[f"A_log_{s}"] = np.tile(np.log(np.arange(1, N + 1, dtype=np.float32)), (DI, 1))
        fake[f"D_{s}"] = np.ones(DI, np.float32)
        fake[f"out_w_{s}"] = rng.standard_normal((DI, DM), dtype=np.float32) * 0.02
    fake["ln_g"] = np.ones(DM, np.float32)
    fake["ln_b"] = np.zeros(DM, np.float32)
    o = kernel(**fake)
    print("out", o.shape, o.dtype, float(np.abs(o).max()))

